# revision 1
# baseline (speedup 1.0000x reference)
"""GNN message-passing (copy_u -> segment mean -> two GEMMs) on 8 trn2 NeuronCores.

Strategy (1D dst partitioning, per sharding hint):
  - Nodes are split into 392 blocks of 128; blocks are dealt to the 8 cores
    sorted by in-edge chunk count so per-position chunk counts (which the
    SPMD program must fix uniformly) stay tight. All in-edges of a node live
    on its owner core.
  - Per block, edges are gathered 128 at a time from HBM via dma_gather
    (h rows, 512B each). int16 gather indices only reach 32768 rows, so h is
    shipped as two tables (rows [0,32767) and [32767,50000)), each with a
    trailing all-zero row used for pad slots.
  - Aggregation per 128-edge chunk: psum[feat,dst] += G_chunk.T @ O_chunk on
    the PE, where O[e,d] = (dst_local[e]==d) * recip_deg[dst_e] is built on
    the DVE with a single fused tensor_scalar (is_equal, mult) per chunk.
    This produces h_N.T (feature-major) with the mean division folded in.
  - Per block: psum_out[dout,node] = W1.T @ hT_blk + W2.T @ hNT_blk
    (weights stationary), bias added during the ScalarE PSUM evacuation
    (activation Identity with per-partition bias). Output is written
    feature-major and transposed back on the host.

Self-contained: only needs numpy + the concourse stack at /opt/trn_rl_repo.
"""

import sys

if "/opt/trn_rl_repo" not in sys.path:
    sys.path.insert(0, "/opt/trn_rl_repo")

import numpy as np
from contextlib import ExitStack

N_NODES = 50000
N_EDGES = 800000
D = 128
P = 128
NCORES = 8
NB = 49                      # blocks per core
NPC = NB * P                 # nodes per core (6272)
NTOT = NPC * NCORES          # padded node count (50176)
SPLIT = 32767                # table A holds h rows [0, SPLIT); idx SPLIT = zero row
B_ROWS = N_NODES - SPLIT     # 17233 data rows in table B
BCH_A = 16                   # chunks per gather batch (2048 idxs, single_packet=False)
BCH_B = 16                   # chunks per gather batch, B stream
PAD_DLOC = 200.0             # dst_local sentinel -> one-hot row of zeros


def _prep(h, src, dst, W1, b1, W2, b2):
    """Host-side scheduling. Returns (in_maps, meta)."""
    src = np.asarray(src).astype(np.int64)
    dst = np.asarray(dst).astype(np.int64)
    h = np.asarray(h, dtype=np.float32)

    deg = np.bincount(dst, minlength=N_NODES).astype(np.float32)
    recip = (1.0 / np.maximum(deg, 1.0)).astype(np.float32)

    tabA = np.vstack([h[:SPLIT], np.zeros((1, D), np.float32)])          # [32768, D]
    tabB = np.vstack([h[SPLIT:], np.zeros((1, D), np.float32)])          # [17234, D]

    gblk = dst // P                                   # global block id, 0..391
    slot = dst % P
    cls = (src >= SPLIT).astype(np.int64)

    # per-global-block class counts -> balanced assignment of blocks to
    # (core, position): sort blocks by chunk needs, deal 8 consecutive per
    # position so the per-position max over cores is tight.
    gcnt = np.bincount(gblk * 2 + cls, minlength=NCORES * NB * 2).reshape(NCORES * NB, 2)
    need = np.ceil(gcnt / P).astype(np.int64)         # [392, 2]
    blk_order = np.lexsort((need[:, 1], need[:, 0]))[::-1]
    asg = blk_order.reshape(NB, NCORES)               # asg[j, c] = global block
    # owner/position lookup per global block
    pos_of = np.empty(NCORES * NB, np.int64)
    core_of = np.empty(NCORES * NB, np.int64)
    for j in range(NB):
        for c in range(NCORES):
            g = asg[j, c]
            pos_of[g] = j
            core_of[g] = c
    owner = core_of[gblk]
    blk = pos_of[gblk]

    # counts per (core, position, class)
    key = ((owner * NB) + blk) * 2 + cls
    cnt = np.bincount(key, minlength=NCORES * NB * 2).reshape(NCORES, NB, 2)

    # program chunk counts per block position (uniform across cores)
    kA = np.maximum(np.ceil(cnt[:, :, 0] / P).astype(np.int64).max(axis=0), 1)  # [NB]
    kB = np.ceil(cnt[:, :, 1] / P).astype(np.int64).max(axis=0)                 # [NB]
    startA = np.concatenate([[0], np.cumsum(kA)])
    startB = np.concatenate([[0], np.cumsum(kB)])
    nchA, nchB = int(startA[-1]), int(startB[-1])
    nchB = max(nchB, 1)  # keep tensor shapes nonzero if no class-B edges exist
    SA, SB = nchA * P, nchB * P

    # per-edge slot position within its (core, blk, cls) group; edges are
    # ordered by src within each group so every gather batch reads ascending
    # HBM addresses (row-buffer/bank friendliness; any order is correct)
    order = np.lexsort((src, cls, blk, owner))
    e_sorted = order
    grp = key[e_sorted]
    # position within group
    grp_change = np.concatenate([[0], np.cumsum(grp[1:] != grp[:-1])])
    first_of_grp = np.concatenate([[0], np.flatnonzero(grp[1:] != grp[:-1]) + 1])
    pos = np.arange(len(e_sorted)) - first_of_grp[grp_change]

    in_maps = []
    start = {0: startA, 1: startB}
    for c in range(NCORES):
        idxs = {0: np.full(SA, SPLIT, np.int64), 1: np.full(SB, B_ROWS, np.int64)}
        dloc = {0: np.full(SA, PAD_DLOC, np.float32), 1: np.full(SB, PAD_DLOC, np.float32)}
        rcp = {0: np.zeros(SA, np.float32), 1: np.zeros(SB, np.float32)}

        mask_c = owner[e_sorted] == c
        for k in (0, 1):
            m = mask_c & (cls[e_sorted] == k)
            es = e_sorted[m]
            ps = pos[m]
            gslot = start[k][blk[es]] * P + ps
            v = src[es] - (SPLIT if k else 0)
            idxs[k][gslot] = v
            dloc[k][gslot] = slot[es]
            rcp[k][gslot] = recip[dst[es]]

        def idx_layout(v):
            w = v.astype(np.int16).reshape(-1, 16).T        # [16, S/16]
            return np.tile(w, (8, 1)).copy()                # [128, S/16]

        def col_layout(v):
            return v.reshape(-1, P).T.copy()                # [128, nch]

        hT = np.zeros((D, NPC), np.float32)
        for j in range(NB):
            g = asg[j, c]
            lo = g * P
            hi = min(lo + P, N_NODES)
            if hi > lo:
                hT[:, j * P : j * P + (hi - lo)] = h[lo:hi].T

        aux0 = np.concatenate(
            [
                np.asarray(W1, np.float32),
                np.asarray(W2, np.float32),
                np.tile(np.arange(P, dtype=np.float32)[None, :], (P, 1)),
                (np.asarray(b1, np.float32) + np.asarray(b2, np.float32))[:, None],
            ],
            axis=1,
        )  # [128, 385]

        in_maps.append(
            {
                "tabA": tabA,
                "tabB": tabB,
                "idxA": idx_layout(idxs[0]),
                "idxB": idx_layout(idxs[1]),
                "dlA": np.concatenate([col_layout(dloc[0]), col_layout(rcp[0])], axis=1),
                "dlB": np.concatenate([col_layout(dloc[1]), col_layout(rcp[1])], axis=1),
                "hT": hT,
                "aux0": aux0,
            }
        )

    meta = dict(
        kA=kA, kB=kB, startA=startA, startB=startB, nchA=nchA, nchB=nchB, asg=asg
    )
    return in_maps, meta


def _build(meta):
    import concourse.bacc as bacc
    import concourse.mybir as mybir
    import concourse.tile as tile

    kA, kB = meta["kA"], meta["kB"]
    startA, startB = meta["startA"], meta["startB"]
    nchA, nchB = meta["nchA"], meta["nchB"]
    f32 = mybir.dt.float32

    nc = bacc.Bacc("TRN2", target_bir_lowering=False, debug=False, num_devices=NCORES)
    tabA = nc.declare_dram_parameter("tabA", [SPLIT + 1, D], f32, isOutput=False)
    tabB = nc.declare_dram_parameter("tabB", [B_ROWS + 1, D], f32, isOutput=False)
    idxA = nc.declare_dram_parameter("idxA", [P, nchA * 8], mybir.dt.int16, isOutput=False)
    idxB = nc.declare_dram_parameter("idxB", [P, nchB * 8], mybir.dt.int16, isOutput=False)
    dlA = nc.declare_dram_parameter("dlA", [P, 2 * nchA], f32, isOutput=False)
    dlB = nc.declare_dram_parameter("dlB", [P, 2 * nchB], f32, isOutput=False)
    hT = nc.declare_dram_parameter("hT", [D, NPC], f32, isOutput=False)
    aux0 = nc.declare_dram_parameter("aux0", [P, 385], f32, isOutput=False)
    outT = nc.declare_dram_parameter("outT", [D, NPC], f32, isOutput=True)

    tabs = {0: tabA, 1: tabB}
    idx_d = {0: idxA, 1: idxB}
    dl_d = {0: dlA, 1: dlB}
    nch = {0: nchA, 1: nchB}
    bch = {0: BCH_A, 1: BCH_B}

    with tile.TileContext(nc) as tc, ExitStack() as ctx:
        consts = ctx.enter_context(tc.tile_pool(name="consts", bufs=1))
        gA_pool = ctx.enter_context(tc.tile_pool(name="gA", bufs=3))
        gB_pool = ctx.enter_context(tc.tile_pool(name="gB", bufs=3))
        oh_pool = ctx.enter_context(tc.tile_pool(name="oh", bufs=6))
        hT_pool = ctx.enter_context(tc.tile_pool(name="hTp", bufs=2))
        hN_pool = ctx.enter_context(tc.tile_pool(name="hNp", bufs=2))
        out_pool = ctx.enter_context(tc.tile_pool(name="outp", bufs=2))
        psA = ctx.enter_context(tc.tile_pool(name="psA", bufs=2, space="PSUM"))
        psO = ctx.enter_context(tc.tile_pool(name="psO", bufs=2, space="PSUM"))

        # constant loads
        idx_t = {}
        dl_t = {}
        for k in (0, 1):
            it = consts.tile([P, nch[k] * 8], mybir.dt.int16, name=f"idx{k}")
            nc.sync.dma_start(it[:], idx_d[k][:])
            idx_t[k] = it
            dt_ = consts.tile([P, 2 * nch[k]], f32, name=f"dl{k}")
            nc.sync.dma_start(dt_[:], dl_d[k][:])
            dl_t[k] = dt_
        aux_t = consts.tile([P, 385], f32)
        nc.sync.dma_start(aux_t[:], aux0[:])
        w1_s = aux_t[:, 0:128]
        w2_s = aux_t[:, 128:256]
        iota_s = aux_t[:, 256:384]
        bias_s = aux_t[:, 384:385]

        g_pool = {0: gA_pool, 1: gB_pool}
        g_state = {0: (-1, None), 1: (-1, None)}

        def chunk_ap(k, j):
            """AP of gathered G chunk j of stream k (emits batch gather on demand)."""
            b, off = divmod(j, bch[k])
            cur, tile_ = g_state[k]
            if b != cur:
                nchunks = min(bch[k], nch[k] - b * bch[k])
                gt = g_pool[k].tile([P, bch[k] * D], f32, name=f"g{k}")
                ni = nchunks * P
                nc.gpsimd.dma_gather(
                    gt[:, : nchunks * D].rearrange("p (c e) -> p c e", e=D),
                    tabs[k][:, :],
                    idx_t[k][:, b * bch[k] * 8 : b * bch[k] * 8 + (ni // 16)],
                    ni,
                    ni,
                    D,
                    single_packet=False,
                )
                g_state[k] = (b, gt)
                tile_ = gt
            return tile_[:, off * D : (off + 1) * D]

        for j in range(NB):
            agg = psA.tile([P, P], f32)
            plan = [(0, int(startA[j]) + t) for t in range(int(kA[j]))] + [
                (1, int(startB[j]) + t) for t in range(int(kB[j]))
            ]
            for i, (k, ch) in enumerate(plan):
                g_ap = chunk_ap(k, ch)
                oh = oh_pool.tile([P, P], f32)
                nc.vector.tensor_scalar(
                    out=oh[:],
                    in0=iota_s,
                    scalar1=dl_t[k][:, ch : ch + 1],
                    scalar2=dl_t[k][:, nch[k] + ch : nch[k] + ch + 1],
                    op0=mybir.AluOpType.is_equal,
                    op1=mybir.AluOpType.mult,
                )
                nc.tensor.matmul(
                    agg[:],
                    lhsT=g_ap,
                    rhs=oh[:],
                    start=(i == 0),
                    stop=(i == len(plan) - 1),
                )

            hN_t = hN_pool.tile([P, P], f32)
            nc.scalar.activation(hN_t[:], agg[:], mybir.ActivationFunctionType.Copy)

            hT_t = hT_pool.tile([P, P], f32)
            nc.sync.dma_start(hT_t[:], hT[:, j * P : (j + 1) * P])

            po = psO.tile([P, P], f32)
            nc.tensor.matmul(po[:], lhsT=w1_s, rhs=hT_t[:], start=True, stop=False)
            nc.tensor.matmul(po[:], lhsT=w2_s, rhs=hN_t[:], start=False, stop=True)

            out_t = out_pool.tile([P, P], f32)
            nc.scalar.activation(
                out_t[:], po[:], mybir.ActivationFunctionType.Identity, bias=bias_s
            )
            nc.sync.dma_start(outT[:, j * P : (j + 1) * P], out_t[:])

    nc.finalize()
    return nc


def kernel(h, src, dst, W1, b1, W2, b2):
    from concourse.bass_utils import run_bass_kernel_spmd

    in_maps, meta = _prep(h, src, dst, W1, b1, W2, b2)
    nc = _build(meta)
    res = run_bass_kernel_spmd(nc, in_maps, list(range(NCORES))).results
    return _assemble(res, meta)


def _assemble(res, meta):
    asg = meta["asg"]
    out = np.zeros((NCORES * NPC, D), np.float32)
    for j in range(NB):
        for c in range(NCORES):
            g = asg[j, c]
            out[g * P : (g + 1) * P] = res[c]["outT"][:, j * P : (j + 1) * P].T
    return out[:N_NODES].astype(np.float32)


def _sim(h, src, dst, W1, b1, W2, b2):
    """Numpy simulation of the exact device program (bookkeeping check)."""
    in_maps, meta = _prep(h, src, dst, W1, b1, W2, b2)
    kA, kB = meta["kA"], meta["kB"]
    startA, startB = meta["startA"], meta["startB"]
    nchA, nchB = meta["nchA"], meta["nchB"]
    outs = []
    for c in range(NCORES):
        m = in_maps[c]
        tabs = {0: m["tabA"], 1: m["tabB"]}
        start = {0: startA, 1: startB}
        kS = {0: kA, 1: kB}
        nch = {0: nchA, 1: nchB}
        # un-layout idx: [128, S/16] -> first 16 rows -> slot i = [i%16, i//16]
        idxs = {}
        dloc = {}
        rcp = {}
        for k, nm_i, nm_d in ((0, "idxA", "dlA"), (1, "idxB", "dlB")):
            w = m[nm_i][:16]
            idxs[k] = w.T.reshape(-1)
            dl = m[nm_d]
            dloc[k] = dl[:, : nch[k]]
            rcp[k] = dl[:, nch[k] :]
        outT = np.zeros((D, NPC), np.float32)
        for j in range(NB):
            agg = np.zeros((P, P), np.float32)
            for k in (0, 1):
                for t in range(int(kS[k][j])):
                    ch = int(start[k][j]) + t
                    gi = idxs[k][ch * P : (ch + 1) * P].astype(np.int64)
                    G = tabs[k][gi]                       # [128 edges, D]
                    O = (dloc[k][:, ch, None] == np.arange(P)[None, :]) * rcp[k][:, ch, None]
                    agg += G.T @ O.astype(np.float32)
            hTj = m["hT"][:, j * P : (j + 1) * P]
            po = (
                np.asarray(W1, np.float32).T @ hTj
                + np.asarray(W2, np.float32).T @ agg
            )
            outT[:, j * P : (j + 1) * P] = po + m["aux0"][:, 384:385]
        outs.append({"outT": outT})
    return _assemble(outs, meta)


if __name__ == "__main__":
    # quick host-side validation against the reference math
    rng = np.random.default_rng(0)
    h = rng.standard_normal((N_NODES, D), dtype=np.float32)
    src = rng.integers(0, N_NODES, N_EDGES)
    dst = rng.integers(0, N_NODES, N_EDGES)
    W1 = rng.standard_normal((D, D), dtype=np.float32) * 0.1
    b1 = rng.standard_normal(D, dtype=np.float32) * 0.1
    W2 = rng.standard_normal((D, D), dtype=np.float32) * 0.1
    b2 = rng.standard_normal(D, dtype=np.float32) * 0.1

    msgs_sum = np.zeros((N_NODES, D), np.float32)
    np.add.at(msgs_sum, dst, h[src])
    deg = np.bincount(dst, minlength=N_NODES).astype(np.float32)
    hN = msgs_sum / np.maximum(deg, 1.0)[:, None]
    ref = h @ W1 + b1 + hN @ W2 + b2

    got = _sim(h, src, dst, W1, b1, W2, b2)
    err = np.abs(got - ref).max() / np.abs(ref).max()
    print("sim rel err:", err)



# revision 4
# speedup vs baseline: 1.1688x; 1.1688x over previous
"""GNN message-passing (copy_u -> segment mean -> two GEMMs) on 8 trn2 NeuronCores.

Strategy (1D dst partitioning, per sharding hint):
  - Nodes are split into 392 blocks of 128; blocks are dealt to the 8 cores
    sorted by in-edge chunk count so per-position chunk counts (which the
    SPMD program must fix uniformly) stay tight. All in-edges of a node live
    on its owner core.
  - h is shipped in bf16. Per block, edges are gathered 128 at a time from
    HBM via dma_gather (256B rows). int16 gather indices only reach 32768
    rows, so h is shipped as two tables (rows [0,32767) and [32767,50000)),
    each with a trailing all-zero row used for pad slots.
  - Aggregation per 128-edge chunk: psum[feat,dst] += G_chunk.T @ O_chunk on
    the PE (bf16: 1 cycle/row vs 4 for fp32), where
    O[e,d] = (dst_local[e]==d) * recip_deg[dst_e] is built on the DVE with a
    single fused tensor_scalar (is_equal, mult) per chunk, in bf16 (2x DVE).
    This produces h_N.T (feature-major) with the mean division folded in.
  - Per block: psum_out[dout,node] = W1.T @ hT_blk + W2.T @ hNT_blk
    (weights stationary, all bf16), bias added during the ScalarE PSUM
    evacuation (activation Identity with per-partition fp32 bias). Output is
    written feature-major in bf16 and transposed back on the host.

Self-contained: only needs numpy + the concourse stack at /opt/trn_rl_repo.
"""

import sys

if "/opt/trn_rl_repo" not in sys.path:
    sys.path.insert(0, "/opt/trn_rl_repo")

import numpy as np
from contextlib import ExitStack

import ml_dtypes

BF16 = ml_dtypes.bfloat16

N_NODES = 50000
N_EDGES = 800000
D = 128
P = 128
NCORES = 8
NB = 49                      # blocks per core
NPC = NB * P                 # nodes per core (6272)
NTOT = NPC * NCORES          # padded node count (50176)
SPLIT = 32767                # table A holds h rows [0, SPLIT); idx SPLIT = zero row
B_ROWS = N_NODES - SPLIT     # 17233 data rows in table B
BCH_A = 32                   # chunks per gather batch (4096 idxs)
BCH_B = 32                   # chunks per gather batch, B stream
PAD_DLOC = 200.0             # dst_local sentinel -> one-hot row of zeros


def _prep(h, src, dst, W1, b1, W2, b2):
    """Host-side scheduling. Returns (in_maps, meta)."""
    src = np.asarray(src).astype(np.int64)
    dst = np.asarray(dst).astype(np.int64)
    h = np.asarray(h, dtype=np.float32)

    deg = np.bincount(dst, minlength=N_NODES).astype(np.float32)
    recip = (1.0 / np.maximum(deg, 1.0)).astype(np.float32)

    h16 = h.astype(BF16)
    tabA = np.vstack([h16[:SPLIT], np.zeros((1, D), BF16)])          # [32768, D]
    tabB = np.vstack([h16[SPLIT:], np.zeros((1, D), BF16)])          # [17234, D]

    gblk = dst // P                                   # global block id, 0..391
    slot = dst % P
    cls = (src >= SPLIT).astype(np.int64)

    # per-global-block class counts -> balanced assignment of blocks to
    # (core, position): sort blocks by chunk needs, deal 8 consecutive per
    # position so the per-position max over cores is tight.
    gcnt = np.bincount(gblk * 2 + cls, minlength=NCORES * NB * 2).reshape(NCORES * NB, 2)
    need = np.ceil(gcnt / P).astype(np.int64)         # [392, 2]
    blk_order = np.lexsort((need[:, 1], need[:, 0]))[::-1]
    asg = blk_order.reshape(NB, NCORES)               # asg[j, c] = global block
    # owner/position lookup per global block
    pos_of = np.empty(NCORES * NB, np.int64)
    core_of = np.empty(NCORES * NB, np.int64)
    for j in range(NB):
        for c in range(NCORES):
            g = asg[j, c]
            pos_of[g] = j
            core_of[g] = c
    owner = core_of[gblk]
    blk = pos_of[gblk]

    # counts per (core, position, class)
    key = ((owner * NB) + blk) * 2 + cls
    cnt = np.bincount(key, minlength=NCORES * NB * 2).reshape(NCORES, NB, 2)

    # program chunk counts per block position (uniform across cores)
    kA = np.maximum(np.ceil(cnt[:, :, 0] / P).astype(np.int64).max(axis=0), 1)  # [NB]
    kB = np.ceil(cnt[:, :, 1] / P).astype(np.int64).max(axis=0)                 # [NB]
    startA = np.concatenate([[0], np.cumsum(kA)])
    startB = np.concatenate([[0], np.cumsum(kB)])
    nchA, nchB = int(startA[-1]), int(startB[-1])
    nchB = max(nchB, 1)  # keep tensor shapes nonzero if no class-B edges exist
    SA, SB = nchA * P, nchB * P

    # per-edge slot position within its (core, blk, cls) group; edges are
    # ordered by src within each group so every gather batch reads ascending
    # HBM addresses (row-buffer/bank friendliness; any order is correct)
    order = np.lexsort((src, cls, blk, owner))
    e_sorted = order
    grp = key[e_sorted]
    # position within group
    grp_change = np.concatenate([[0], np.cumsum(grp[1:] != grp[:-1])])
    first_of_grp = np.concatenate([[0], np.flatnonzero(grp[1:] != grp[:-1]) + 1])
    pos = np.arange(len(e_sorted)) - first_of_grp[grp_change]

    in_maps = []
    start = {0: startA, 1: startB}
    for c in range(NCORES):
        idxs = {0: np.full(SA, SPLIT, np.int64), 1: np.full(SB, B_ROWS, np.int64)}
        dloc = {0: np.full(SA, PAD_DLOC, np.float32), 1: np.full(SB, PAD_DLOC, np.float32)}
        rcp = {0: np.zeros(SA, np.float32), 1: np.zeros(SB, np.float32)}

        mask_c = owner[e_sorted] == c
        for k in (0, 1):
            m = mask_c & (cls[e_sorted] == k)
            es = e_sorted[m]
            ps = pos[m]
            gslot = start[k][blk[es]] * P + ps
            v = src[es] - (SPLIT if k else 0)
            idxs[k][gslot] = v
            dloc[k][gslot] = slot[es]
            rcp[k][gslot] = recip[dst[es]]

        def idx_layout(v):
            w = v.astype(np.int16).reshape(-1, 16).T        # [16, S/16]
            return np.tile(w, (8, 1)).copy()                # [128, S/16]

        def col_layout(v):
            return v.reshape(-1, P).T.copy()                # [128, nch]

        hT = np.zeros((D, NPC), np.float32)
        for j in range(NB):
            g = asg[j, c]
            lo = g * P
            hi = min(lo + P, N_NODES)
            if hi > lo:
                hT[:, j * P : j * P + (hi - lo)] = h[lo:hi].T

        # bf16 constants: W1 | W2 | iota  (dst_local values & iota are exact)
        auxW = np.concatenate(
            [
                np.asarray(W1, np.float32),
                np.asarray(W2, np.float32),
                np.tile(np.arange(P, dtype=np.float32)[None, :], (P, 1)),
            ],
            axis=1,
        ).astype(BF16)  # [128, 384]
        auxB = (np.asarray(b1, np.float32) + np.asarray(b2, np.float32))[:, None]

        in_maps.append(
            {
                "tabA": tabA,
                "tabB": tabB,
                "idxA": idx_layout(idxs[0]),
                "idxB": idx_layout(idxs[1]),
                "dlA": np.concatenate([col_layout(dloc[0]), col_layout(rcp[0])], axis=1),
                "dlB": np.concatenate([col_layout(dloc[1]), col_layout(rcp[1])], axis=1),
                "hT": hT.astype(BF16),
                "auxW": auxW,
                "auxB": auxB.astype(np.float32),
            }
        )

    meta = dict(
        kA=kA, kB=kB, startA=startA, startB=startB, nchA=nchA, nchB=nchB, asg=asg
    )
    return in_maps, meta


def _build(meta):
    import concourse.bacc as bacc
    import concourse.mybir as mybir
    import concourse.tile as tile

    kA, kB = meta["kA"], meta["kB"]
    startA, startB = meta["startA"], meta["startB"]
    nchA, nchB = meta["nchA"], meta["nchB"]
    f32 = mybir.dt.float32
    bf16 = mybir.dt.bfloat16

    nc = bacc.Bacc("TRN2", target_bir_lowering=False, debug=False, num_devices=NCORES)
    tabA = nc.declare_dram_parameter("tabA", [SPLIT + 1, D], bf16, isOutput=False)
    tabB = nc.declare_dram_parameter("tabB", [B_ROWS + 1, D], bf16, isOutput=False)
    idxA = nc.declare_dram_parameter("idxA", [P, nchA * 8], mybir.dt.int16, isOutput=False)
    idxB = nc.declare_dram_parameter("idxB", [P, nchB * 8], mybir.dt.int16, isOutput=False)
    dlA = nc.declare_dram_parameter("dlA", [P, 2 * nchA], f32, isOutput=False)
    dlB = nc.declare_dram_parameter("dlB", [P, 2 * nchB], f32, isOutput=False)
    hT = nc.declare_dram_parameter("hT", [D, NPC], bf16, isOutput=False)
    auxW = nc.declare_dram_parameter("auxW", [P, 384], bf16, isOutput=False)
    auxB = nc.declare_dram_parameter("auxB", [P, 1], f32, isOutput=False)
    outT = nc.declare_dram_parameter("outT", [D, NPC], bf16, isOutput=True)

    tabs = {0: tabA, 1: tabB}
    idx_d = {0: idxA, 1: idxB}
    dl_d = {0: dlA, 1: dlB}
    nch = {0: nchA, 1: nchB}
    bch = {0: BCH_A, 1: BCH_B}

    with tile.TileContext(nc) as tc, ExitStack() as ctx:
        consts = ctx.enter_context(tc.tile_pool(name="consts", bufs=1))
        gA_pool = ctx.enter_context(tc.tile_pool(name="gA", bufs=3))
        gB_pool = ctx.enter_context(tc.tile_pool(name="gB", bufs=3))
        oh_pool = ctx.enter_context(tc.tile_pool(name="oh", bufs=6))
        hT_pool = ctx.enter_context(tc.tile_pool(name="hTp", bufs=2))
        hN_pool = ctx.enter_context(tc.tile_pool(name="hNp", bufs=2))
        out_pool = ctx.enter_context(tc.tile_pool(name="outp", bufs=3))
        psA = ctx.enter_context(tc.tile_pool(name="psA", bufs=2, space="PSUM"))
        psO = ctx.enter_context(tc.tile_pool(name="psO", bufs=2, space="PSUM"))

        # constant loads
        idx_t = {}
        dl_t = {}
        for k in (0, 1):
            it = consts.tile([P, nch[k] * 8], mybir.dt.int16, name=f"idx{k}")
            nc.sync.dma_start(it[:], idx_d[k][:])
            idx_t[k] = it
            dt_ = consts.tile([P, 2 * nch[k]], f32, name=f"dl{k}")
            nc.sync.dma_start(dt_[:], dl_d[k][:])
            dl_t[k] = dt_
        auxW_t = consts.tile([P, 384], bf16)
        nc.sync.dma_start(auxW_t[:], auxW[:])
        auxB_t = consts.tile([P, 1], f32)
        nc.sync.dma_start(auxB_t[:], auxB[:])
        w1_s = auxW_t[:, 0:128]
        w2_s = auxW_t[:, 128:256]
        iota_s = auxW_t[:, 256:384]
        bias_s = auxB_t[:, 0:1]

        g_pool = {0: gA_pool, 1: gB_pool}
        g_state = {0: (-1, None), 1: (-1, None)}

        def chunk_ap(k, j):
            """AP of gathered G chunk j of stream k (emits batch gather on demand)."""
            b, off = divmod(j, bch[k])
            cur, tile_ = g_state[k]
            if b != cur:
                nchunks = min(bch[k], nch[k] - b * bch[k])
                gt = g_pool[k].tile([P, bch[k] * D], bf16, name=f"g{k}")
                ni = nchunks * P
                nc.gpsimd.dma_gather(
                    gt[:, : nchunks * D].rearrange("p (c e) -> p c e", e=D),
                    tabs[k][:, :],
                    idx_t[k][:, b * bch[k] * 8 : b * bch[k] * 8 + (ni // 16)],
                    ni,
                    ni,
                    D,
                    single_packet=False,
                )
                g_state[k] = (b, gt)
                tile_ = gt
            return tile_[:, off * D : (off + 1) * D]

        for j in range(NB):
            agg = psA.tile([P, P], f32)
            plan = [(0, int(startA[j]) + t) for t in range(int(kA[j]))] + [
                (1, int(startB[j]) + t) for t in range(int(kB[j]))
            ]
            for i, (k, ch) in enumerate(plan):
                g_ap = chunk_ap(k, ch)
                oh = oh_pool.tile([P, P], bf16)
                nc.vector.tensor_scalar(
                    out=oh[:],
                    in0=iota_s,
                    scalar1=dl_t[k][:, ch : ch + 1],
                    scalar2=dl_t[k][:, nch[k] + ch : nch[k] + ch + 1],
                    op0=mybir.AluOpType.is_equal,
                    op1=mybir.AluOpType.mult,
                )
                nc.tensor.matmul(
                    agg[:],
                    lhsT=g_ap,
                    rhs=oh[:],
                    start=(i == 0),
                    stop=(i == len(plan) - 1),
                )

            hN_t = hN_pool.tile([P, P], bf16)
            nc.scalar.activation(hN_t[:], agg[:], mybir.ActivationFunctionType.Copy)

            if j % 2 == 0:
                hT_t2 = hT_pool.tile([P, 2 * P], bf16)
                hi = min(2 * P, NPC - j * P)
                nc.sync.dma_start(hT_t2[:, :hi], hT[:, j * P : j * P + hi])
            hT_slice = hT_t2[:, 0:P] if j % 2 == 0 else hT_t2[:, P : 2 * P]

            po = psO.tile([P, P], f32)
            nc.tensor.matmul(po[:], lhsT=w1_s, rhs=hT_slice, start=True, stop=False)
            nc.tensor.matmul(po[:], lhsT=w2_s, rhs=hN_t[:], start=False, stop=True)

            out_t = out_pool.tile([P, P], bf16)
            nc.scalar.activation(
                out_t[:], po[:], mybir.ActivationFunctionType.Identity, bias=bias_s
            )
            nc.sync.dma_start(outT[:, j * P : (j + 1) * P], out_t[:])

    nc.finalize()
    return nc


def kernel(h, src, dst, W1, b1, W2, b2):
    from concourse.bass_utils import run_bass_kernel_spmd

    in_maps, meta = _prep(h, src, dst, W1, b1, W2, b2)
    nc = _build(meta)
    res = run_bass_kernel_spmd(nc, in_maps, list(range(NCORES))).results
    return _assemble(res, meta)


def _assemble(res, meta):
    asg = meta["asg"]
    out = np.zeros((NCORES * NPC, D), np.float32)
    for j in range(NB):
        for c in range(NCORES):
            g = asg[j, c]
            out[g * P : (g + 1) * P] = (
                res[c]["outT"][:, j * P : (j + 1) * P].astype(np.float32).T
            )
    return out[:N_NODES].astype(np.float32)


def _sim(h, src, dst, W1, b1, W2, b2):
    """Numpy simulation of the exact device program (bookkeeping check)."""
    in_maps, meta = _prep(h, src, dst, W1, b1, W2, b2)
    kA, kB = meta["kA"], meta["kB"]
    startA, startB = meta["startA"], meta["startB"]
    nchA, nchB = meta["nchA"], meta["nchB"]
    outs = []
    for c in range(NCORES):
        m = in_maps[c]
        tabs = {0: m["tabA"], 1: m["tabB"]}
        start = {0: startA, 1: startB}
        kS = {0: kA, 1: kB}
        nch = {0: nchA, 1: nchB}
        # un-layout idx: [128, S/16] -> first 16 rows -> slot i = [i%16, i//16]
        idxs = {}
        dloc = {}
        rcp = {}
        for k, nm_i, nm_d in ((0, "idxA", "dlA"), (1, "idxB", "dlB")):
            w = m[nm_i][:16]
            idxs[k] = w.T.reshape(-1)
            dl = m[nm_d].astype(np.float32)
            dloc[k] = dl[:, : nch[k]]
            rcp[k] = dl[:, nch[k] :]
        W1b = m["auxW"][:, 0:128].astype(np.float32)
        W2b = m["auxW"][:, 128:256].astype(np.float32)
        outT = np.zeros((D, NPC), np.float32)
        for j in range(NB):
            agg = np.zeros((P, P), np.float32)
            for k in (0, 1):
                for t in range(int(kS[k][j])):
                    ch = int(start[k][j]) + t
                    gi = idxs[k][ch * P : (ch + 1) * P].astype(np.int64)
                    G = tabs[k][gi].astype(np.float32)     # [128 edges, D]
                    O = (dloc[k][:, ch, None] == np.arange(P)[None, :]) * rcp[k][
                        :, ch, None
                    ]
                    O = O.astype(BF16).astype(np.float32)
                    agg += G.T @ O
            hTj = m["hT"][:, j * P : (j + 1) * P].astype(np.float32)
            hNj = agg.astype(BF16).astype(np.float32)
            po = W1b.T @ hTj + W2b.T @ hNj
            outT[:, j * P : (j + 1) * P] = (po + m["auxB"]).astype(BF16)
        outs.append({"outT": outT.astype(BF16)})
    return _assemble(outs, meta)


if __name__ == "__main__":
    # quick host-side validation against the reference math
    rng = np.random.default_rng(0)
    h = rng.standard_normal((N_NODES, D), dtype=np.float32)
    src = rng.integers(0, N_NODES, N_EDGES)
    dst = rng.integers(0, N_NODES, N_EDGES)
    W1 = rng.standard_normal((D, D), dtype=np.float32) * 0.1
    b1 = rng.standard_normal(D, dtype=np.float32) * 0.1
    W2 = rng.standard_normal((D, D), dtype=np.float32) * 0.1
    b2 = rng.standard_normal(D, dtype=np.float32) * 0.1

    msgs_sum = np.zeros((N_NODES, D), np.float32)
    np.add.at(msgs_sum, dst, h[src])
    deg = np.bincount(dst, minlength=N_NODES).astype(np.float32)
    hN = msgs_sum / np.maximum(deg, 1.0)[:, None]
    ref = h @ W1 + b1 + hN @ W2 + b2

    got = _sim(h, src, dst, W1, b1, W2, b2)
    err = np.linalg.norm(got - ref) / np.linalg.norm(ref)
    print("sim rel err (norm):", err)
    print("sim max abs err:", np.abs(got - ref).max())


# revision 6
# speedup vs baseline: 1.1754x; 1.0056x over previous
"""GNN message-passing (copy_u -> segment mean -> two GEMMs) on 8 trn2 NeuronCores.

Strategy (1D dst partitioning, per sharding hint):
  - Nodes are split into 392 blocks of 128; blocks are dealt to the 8 cores
    sorted by in-edge chunk count so per-position chunk counts (which the
    SPMD program must fix uniformly) stay tight. All in-edges of a node live
    on its owner core.
  - h is shipped in bf16. Per block, edges are gathered 128 at a time from
    HBM via dma_gather (256B rows). int16 gather indices only reach 32768
    rows, so h is shipped as two tables (rows [0,32767) and [32767,50000)),
    each with a trailing all-zero row used for pad slots.
  - Aggregation per 128-edge chunk: psum[feat,dst] += G_chunk.T @ O_chunk on
    the PE (bf16: 1 cycle/row vs 4 for fp32), where
    O[e,d] = (dst_local[e]==d) * recip_deg[dst_e] is built on the DVE with a
    single fused tensor_scalar (is_equal, mult) per chunk, in bf16 (2x DVE).
    This produces h_N.T (feature-major) with the mean division folded in.
  - Per block: psum_out[dout,node] = W1.T @ hT_blk + W2.T @ hNT_blk
    (weights stationary, all bf16), bias added during the ScalarE PSUM
    evacuation (activation Identity with per-partition fp32 bias). Output is
    written feature-major in bf16 and transposed back on the host.

Self-contained: only needs numpy + the concourse stack at /opt/trn_rl_repo.
"""

import sys

if "/opt/trn_rl_repo" not in sys.path:
    sys.path.insert(0, "/opt/trn_rl_repo")

import numpy as np
from contextlib import ExitStack

import ml_dtypes

BF16 = ml_dtypes.bfloat16

N_NODES = 50000
N_EDGES = 800000
D = 128
P = 128
NCORES = 8
NB = 49                      # blocks per core
NPC = NB * P                 # nodes per core (6272)
NTOT = NPC * NCORES          # padded node count (50176)
SPLIT = 32767                # table A holds h rows [0, SPLIT); idx SPLIT = zero row
B_ROWS = N_NODES - SPLIT     # 17233 data rows in table B
BCH_A = 32                   # chunks per gather batch (4096 idxs)
BCH_B = 32                   # chunks per gather batch, B stream
PAD_DLOC = 200.0             # dst_local sentinel -> one-hot row of zeros


def _prep(h, src, dst, W1, b1, W2, b2):
    """Host-side scheduling. Returns (in_maps, meta)."""
    src = np.asarray(src).astype(np.int64)
    dst = np.asarray(dst).astype(np.int64)
    h = np.asarray(h, dtype=np.float32)

    deg = np.bincount(dst, minlength=N_NODES).astype(np.float32)
    recip = (1.0 / np.maximum(deg, 1.0)).astype(np.float32)

    h16 = h.astype(BF16)
    tabA = np.vstack([h16[:SPLIT], np.zeros((1, D), BF16)])          # [32768, D]
    tabB = np.vstack([h16[SPLIT:], np.zeros((1, D), BF16)])          # [17234, D]

    gblk = dst // P                                   # global block id, 0..391
    slot = dst % P
    cls = (src >= SPLIT).astype(np.int64)

    # per-global-block class counts -> balanced assignment of blocks to
    # (core, position): sort blocks by chunk needs, deal 8 consecutive per
    # position so the per-position max over cores is tight.
    gcnt = np.bincount(gblk * 2 + cls, minlength=NCORES * NB * 2).reshape(NCORES * NB, 2)
    need = np.ceil(gcnt / P).astype(np.int64)         # [392, 2]
    blk_order = np.lexsort((need[:, 1], need[:, 0]))[::-1]
    asg = blk_order.reshape(NB, NCORES)               # asg[j, c] = global block
    # owner/position lookup per global block
    pos_of = np.empty(NCORES * NB, np.int64)
    core_of = np.empty(NCORES * NB, np.int64)
    for j in range(NB):
        for c in range(NCORES):
            g = asg[j, c]
            pos_of[g] = j
            core_of[g] = c
    owner = core_of[gblk]
    blk = pos_of[gblk]

    # counts per (core, position, class)
    key = ((owner * NB) + blk) * 2 + cls
    cnt = np.bincount(key, minlength=NCORES * NB * 2).reshape(NCORES, NB, 2)

    # program chunk counts per block position (uniform across cores)
    kA = np.maximum(np.ceil(cnt[:, :, 0] / P).astype(np.int64).max(axis=0), 1)  # [NB]
    kB = np.ceil(cnt[:, :, 1] / P).astype(np.int64).max(axis=0)                 # [NB]
    startA = np.concatenate([[0], np.cumsum(kA)])
    startB = np.concatenate([[0], np.cumsum(kB)])
    nchA, nchB = int(startA[-1]), int(startB[-1])
    nchB = max(nchB, 1)  # keep tensor shapes nonzero if no class-B edges exist
    SA, SB = nchA * P, nchB * P

    # per-edge slot position within its (core, blk, cls) group; edges are
    # ordered by src within each group so every gather batch reads ascending
    # HBM addresses (row-buffer/bank friendliness; any order is correct)
    order = np.lexsort((src, cls, blk, owner))
    e_sorted = order
    grp = key[e_sorted]
    # position within group
    grp_change = np.concatenate([[0], np.cumsum(grp[1:] != grp[:-1])])
    first_of_grp = np.concatenate([[0], np.flatnonzero(grp[1:] != grp[:-1]) + 1])
    pos = np.arange(len(e_sorted)) - first_of_grp[grp_change]

    in_maps = []
    start = {0: startA, 1: startB}
    for c in range(NCORES):
        idxs = {0: np.full(SA, SPLIT, np.int64), 1: np.full(SB, B_ROWS, np.int64)}
        dloc = {0: np.full(SA, PAD_DLOC, np.float32), 1: np.full(SB, PAD_DLOC, np.float32)}
        rcp = {0: np.zeros(SA, np.float32), 1: np.zeros(SB, np.float32)}

        mask_c = owner[e_sorted] == c
        for k in (0, 1):
            m = mask_c & (cls[e_sorted] == k)
            es = e_sorted[m]
            ps = pos[m]
            gslot = start[k][blk[es]] * P + ps
            v = src[es] - (SPLIT if k else 0)
            idxs[k][gslot] = v
            dloc[k][gslot] = slot[es]
            rcp[k][gslot] = recip[dst[es]]

        def idx_layout(v):
            w = v.astype(np.int16).reshape(-1, 16).T        # [16, S/16]
            return np.tile(w, (8, 1)).copy()                # [128, S/16]

        def col_layout(v):
            return v.reshape(-1, P).T.copy()                # [128, nch]

        hT = np.zeros((D, NPC), np.float32)
        for j in range(NB):
            g = asg[j, c]
            lo = g * P
            hi = min(lo + P, N_NODES)
            if hi > lo:
                hT[:, j * P : j * P + (hi - lo)] = h[lo:hi].T

        # bf16 constants: W1 | W2 | iota  (dst_local values & iota are exact)
        auxW = np.concatenate(
            [
                np.asarray(W1, np.float32),
                np.asarray(W2, np.float32),
                np.tile(np.arange(P, dtype=np.float32)[None, :], (P, 1)),
            ],
            axis=1,
        ).astype(BF16)  # [128, 384]
        auxB = (np.asarray(b1, np.float32) + np.asarray(b2, np.float32))[:, None]

        in_maps.append(
            {
                "tabA": tabA,
                "tabB": tabB,
                "idxA": idx_layout(idxs[0]),
                "idxB": idx_layout(idxs[1]),
                "dlA": np.concatenate([col_layout(dloc[0]), col_layout(rcp[0])], axis=1),
                "dlB": np.concatenate([col_layout(dloc[1]), col_layout(rcp[1])], axis=1),
                "hT": hT.astype(BF16),
                "auxW": auxW,
                "auxB": auxB.astype(np.float32),
            }
        )

    meta = dict(
        kA=kA, kB=kB, startA=startA, startB=startB, nchA=nchA, nchB=nchB, asg=asg
    )
    return in_maps, meta


def _build(meta):
    import concourse.bacc as bacc
    import concourse.mybir as mybir
    import concourse.tile as tile

    kA, kB = meta["kA"], meta["kB"]
    startA, startB = meta["startA"], meta["startB"]
    nchA, nchB = meta["nchA"], meta["nchB"]
    f32 = mybir.dt.float32
    bf16 = mybir.dt.bfloat16

    nc = bacc.Bacc("TRN2", target_bir_lowering=False, debug=False, num_devices=NCORES)
    tabA = nc.declare_dram_parameter("tabA", [SPLIT + 1, D], bf16, isOutput=False)
    tabB = nc.declare_dram_parameter("tabB", [B_ROWS + 1, D], bf16, isOutput=False)
    idxA = nc.declare_dram_parameter("idxA", [P, nchA * 8], mybir.dt.int16, isOutput=False)
    idxB = nc.declare_dram_parameter("idxB", [P, nchB * 8], mybir.dt.int16, isOutput=False)
    dlA = nc.declare_dram_parameter("dlA", [P, 2 * nchA], f32, isOutput=False)
    dlB = nc.declare_dram_parameter("dlB", [P, 2 * nchB], f32, isOutput=False)
    hT = nc.declare_dram_parameter("hT", [D, NPC], bf16, isOutput=False)
    auxW = nc.declare_dram_parameter("auxW", [P, 384], bf16, isOutput=False)
    auxB = nc.declare_dram_parameter("auxB", [P, 1], f32, isOutput=False)
    outT = nc.declare_dram_parameter("outT", [D, NPC], bf16, isOutput=True)

    tabs = {0: tabA, 1: tabB}
    idx_d = {0: idxA, 1: idxB}
    dl_d = {0: dlA, 1: dlB}
    nch = {0: nchA, 1: nchB}
    bch = {0: BCH_A, 1: BCH_B}

    with tile.TileContext(nc) as tc, ExitStack() as ctx:
        consts = ctx.enter_context(tc.tile_pool(name="consts", bufs=1))
        gA_pool = ctx.enter_context(tc.tile_pool(name="gA", bufs=3))
        gB_pool = ctx.enter_context(tc.tile_pool(name="gB", bufs=3))
        oh_pool = ctx.enter_context(tc.tile_pool(name="oh", bufs=6))
        hT_pool = ctx.enter_context(tc.tile_pool(name="hTp", bufs=2))
        hN_pool = ctx.enter_context(tc.tile_pool(name="hNp", bufs=2))
        out_pool = ctx.enter_context(tc.tile_pool(name="outp", bufs=3))
        psA = ctx.enter_context(tc.tile_pool(name="psA", bufs=2, space="PSUM"))
        psO = ctx.enter_context(tc.tile_pool(name="psO", bufs=2, space="PSUM"))

        # constant loads; head slices land first so the gather/onehot pipeline
        # can start while the bulk loads behind it.
        HEAD = 2 * BCH_A
        idx_t = {}
        dl_t = {}
        for k in (0, 1):
            it = consts.tile([P, nch[k] * 8], mybir.dt.int16, name=f"idx{k}")
            h8 = min(HEAD * 8, nch[k] * 8)
            nc.sync.dma_start(it[:, :h8], idx_d[k][:, :h8])
            if nch[k] * 8 > h8:
                nc.sync.dma_start(it[:, h8:], idx_d[k][:, h8:])
            idx_t[k] = it
            dt_ = consts.tile([P, 2 * nch[k]], f32, name=f"dl{k}")
            hd = min(HEAD, nch[k])
            nc.sync.dma_start(dt_[:, :hd], dl_d[k][:, :hd])
            nc.sync.dma_start(
                dt_[:, nch[k] : nch[k] + hd], dl_d[k][:, nch[k] : nch[k] + hd]
            )
            if nch[k] > hd:
                nc.sync.dma_start(dt_[:, hd : nch[k]], dl_d[k][:, hd : nch[k]])
                nc.sync.dma_start(dt_[:, nch[k] + hd :], dl_d[k][:, nch[k] + hd :])
            dl_t[k] = dt_
        auxW_t = consts.tile([P, 384], bf16)
        nc.sync.dma_start(auxW_t[:], auxW[:])
        auxB_t = consts.tile([P, 1], f32)
        nc.sync.dma_start(auxB_t[:], auxB[:])
        w1_s = auxW_t[:, 0:128]
        w2_s = auxW_t[:, 128:256]
        iota_s = auxW_t[:, 256:384]
        bias_s = auxB_t[:, 0:1]

        g_pool = {0: gA_pool, 1: gB_pool}
        g_state = {0: (-1, None), 1: (-1, None)}

        def chunk_ap(k, j):
            """AP of gathered G chunk j of stream k (emits batch gather on demand)."""
            b, off = divmod(j, bch[k])
            cur, tile_ = g_state[k]
            if b != cur:
                nchunks = min(bch[k], nch[k] - b * bch[k])
                gt = g_pool[k].tile([P, bch[k] * D], bf16, name=f"g{k}")
                ni = nchunks * P
                nc.gpsimd.dma_gather(
                    gt[:, : nchunks * D].rearrange("p (c e) -> p c e", e=D),
                    tabs[k][:, :],
                    idx_t[k][:, b * bch[k] * 8 : b * bch[k] * 8 + (ni // 16)],
                    ni,
                    ni,
                    D,
                    single_packet=False,
                )
                g_state[k] = (b, gt)
                tile_ = gt
            return tile_[:, off * D : (off + 1) * D]

        for j in range(NB):
            agg = psA.tile([P, P], f32)
            plan = [(0, int(startA[j]) + t) for t in range(int(kA[j]))] + [
                (1, int(startB[j]) + t) for t in range(int(kB[j]))
            ]
            for i, (k, ch) in enumerate(plan):
                g_ap = chunk_ap(k, ch)
                oh = oh_pool.tile([P, P], bf16)
                nc.vector.tensor_scalar(
                    out=oh[:],
                    in0=iota_s,
                    scalar1=dl_t[k][:, ch : ch + 1],
                    scalar2=dl_t[k][:, nch[k] + ch : nch[k] + ch + 1],
                    op0=mybir.AluOpType.is_equal,
                    op1=mybir.AluOpType.mult,
                )
                nc.tensor.matmul(
                    agg[:],
                    lhsT=g_ap,
                    rhs=oh[:],
                    start=(i == 0),
                    stop=(i == len(plan) - 1),
                )

            hN_t = hN_pool.tile([P, P], bf16)
            nc.scalar.activation(hN_t[:], agg[:], mybir.ActivationFunctionType.Copy)

            if j % 2 == 0:
                hT_t2 = hT_pool.tile([P, 2 * P], bf16)
                hi = min(2 * P, NPC - j * P)
                nc.sync.dma_start(hT_t2[:, :hi], hT[:, j * P : j * P + hi])
            hT_slice = hT_t2[:, 0:P] if j % 2 == 0 else hT_t2[:, P : 2 * P]

            po = psO.tile([P, P], f32)
            nc.tensor.matmul(po[:], lhsT=w1_s, rhs=hT_slice, start=True, stop=False)
            nc.tensor.matmul(po[:], lhsT=w2_s, rhs=hN_t[:], start=False, stop=True)

            if j % 2 == 0:
                out_t2 = out_pool.tile([P, 2 * P], bf16)
            out_sl = out_t2[:, 0:P] if j % 2 == 0 else out_t2[:, P : 2 * P]
            nc.scalar.activation(
                out_sl, po[:], mybir.ActivationFunctionType.Identity, bias=bias_s
            )
            if j % 2 == 1:
                nc.sync.dma_start(outT[:, (j - 1) * P : (j + 1) * P], out_t2[:])
            elif j == NB - 1:
                nc.sync.dma_start(outT[:, j * P : (j + 1) * P], out_t2[:, 0:P])

    nc.finalize()
    return nc


def kernel(h, src, dst, W1, b1, W2, b2):
    from concourse.bass_utils import run_bass_kernel_spmd

    in_maps, meta = _prep(h, src, dst, W1, b1, W2, b2)
    nc = _build(meta)
    res = run_bass_kernel_spmd(nc, in_maps, list(range(NCORES))).results
    return _assemble(res, meta)


def _assemble(res, meta):
    asg = meta["asg"]
    out = np.zeros((NCORES * NPC, D), np.float32)
    for j in range(NB):
        for c in range(NCORES):
            g = asg[j, c]
            out[g * P : (g + 1) * P] = (
                res[c]["outT"][:, j * P : (j + 1) * P].astype(np.float32).T
            )
    return out[:N_NODES].astype(np.float32)


def _sim(h, src, dst, W1, b1, W2, b2):
    """Numpy simulation of the exact device program (bookkeeping check)."""
    in_maps, meta = _prep(h, src, dst, W1, b1, W2, b2)
    kA, kB = meta["kA"], meta["kB"]
    startA, startB = meta["startA"], meta["startB"]
    nchA, nchB = meta["nchA"], meta["nchB"]
    outs = []
    for c in range(NCORES):
        m = in_maps[c]
        tabs = {0: m["tabA"], 1: m["tabB"]}
        start = {0: startA, 1: startB}
        kS = {0: kA, 1: kB}
        nch = {0: nchA, 1: nchB}
        # un-layout idx: [128, S/16] -> first 16 rows -> slot i = [i%16, i//16]
        idxs = {}
        dloc = {}
        rcp = {}
        for k, nm_i, nm_d in ((0, "idxA", "dlA"), (1, "idxB", "dlB")):
            w = m[nm_i][:16]
            idxs[k] = w.T.reshape(-1)
            dl = m[nm_d].astype(np.float32)
            dloc[k] = dl[:, : nch[k]]
            rcp[k] = dl[:, nch[k] :]
        W1b = m["auxW"][:, 0:128].astype(np.float32)
        W2b = m["auxW"][:, 128:256].astype(np.float32)
        outT = np.zeros((D, NPC), np.float32)
        for j in range(NB):
            agg = np.zeros((P, P), np.float32)
            for k in (0, 1):
                for t in range(int(kS[k][j])):
                    ch = int(start[k][j]) + t
                    gi = idxs[k][ch * P : (ch + 1) * P].astype(np.int64)
                    G = tabs[k][gi].astype(np.float32)     # [128 edges, D]
                    O = (dloc[k][:, ch, None] == np.arange(P)[None, :]) * rcp[k][
                        :, ch, None
                    ]
                    O = O.astype(BF16).astype(np.float32)
                    agg += G.T @ O
            hTj = m["hT"][:, j * P : (j + 1) * P].astype(np.float32)
            hNj = agg.astype(BF16).astype(np.float32)
            po = W1b.T @ hTj + W2b.T @ hNj
            outT[:, j * P : (j + 1) * P] = (po + m["auxB"]).astype(BF16)
        outs.append({"outT": outT.astype(BF16)})
    return _assemble(outs, meta)


if __name__ == "__main__":
    # quick host-side validation against the reference math
    rng = np.random.default_rng(0)
    h = rng.standard_normal((N_NODES, D), dtype=np.float32)
    src = rng.integers(0, N_NODES, N_EDGES)
    dst = rng.integers(0, N_NODES, N_EDGES)
    W1 = rng.standard_normal((D, D), dtype=np.float32) * 0.1
    b1 = rng.standard_normal(D, dtype=np.float32) * 0.1
    W2 = rng.standard_normal((D, D), dtype=np.float32) * 0.1
    b2 = rng.standard_normal(D, dtype=np.float32) * 0.1

    msgs_sum = np.zeros((N_NODES, D), np.float32)
    np.add.at(msgs_sum, dst, h[src])
    deg = np.bincount(dst, minlength=N_NODES).astype(np.float32)
    hN = msgs_sum / np.maximum(deg, 1.0)[:, None]
    ref = h @ W1 + b1 + hN @ W2 + b2

    got = _sim(h, src, dst, W1, b1, W2, b2)
    err = np.linalg.norm(got - ref) / np.linalg.norm(ref)
    print("sim rel err (norm):", err)
    print("sim max abs err:", np.abs(got - ref).max())


# revision 8
# speedup vs baseline: 1.1805x; 1.0044x over previous
"""GNN message-passing (copy_u -> segment mean -> two GEMMs) on 8 trn2 NeuronCores.

Strategy (1D dst partitioning, per sharding hint):
  - Nodes are split into 392 blocks of 128; blocks are dealt to the 8 cores
    sorted by in-edge chunk count so per-position chunk counts (which the
    SPMD program must fix uniformly) stay tight. All in-edges of a node live
    on its owner core.
  - h is shipped in bf16. Per block, edges are gathered 128 at a time from
    HBM via dma_gather (256B rows). int16 gather indices only reach 32768
    rows, so h is shipped as two tables (rows [0,32767) and [32767,50000)),
    each with a trailing all-zero row used for pad slots.
  - Aggregation per 128-edge chunk: psum[feat,dst] += G_chunk.T @ O_chunk on
    the PE (bf16: 1 cycle/row vs 4 for fp32), where
    O[e,d] = (dst_local[e]==d) * recip_deg[dst_e] is built on the DVE with a
    single fused tensor_scalar (is_equal, mult) per chunk, in bf16 (2x DVE).
    This produces h_N.T (feature-major) with the mean division folded in.
  - Per block: psum_out[dout,node] = W1.T @ hT_blk + W2.T @ hNT_blk
    (weights stationary, all bf16), bias added during the ScalarE PSUM
    evacuation (activation Identity with per-partition fp32 bias). Output is
    written feature-major in bf16 and transposed back on the host.

Self-contained: only needs numpy + the concourse stack at /opt/trn_rl_repo.
"""

import sys

if "/opt/trn_rl_repo" not in sys.path:
    sys.path.insert(0, "/opt/trn_rl_repo")

import numpy as np
from contextlib import ExitStack

import ml_dtypes

BF16 = ml_dtypes.bfloat16

N_NODES = 50000
N_EDGES = 800000
D = 128
P = 128
NCORES = 8
NB = 49                      # blocks per core
NPC = NB * P                 # nodes per core (6272)
NTOT = NPC * NCORES          # padded node count (50176)
SPLIT = 32767                # table A holds h rows [0, SPLIT); idx SPLIT = zero row
B_ROWS = N_NODES - SPLIT     # 17233 data rows in table B
BCH_A = 32                   # chunks per gather batch (4096 idxs)
BCH_B = 32                   # chunks per gather batch, B stream
PAD_DLOC = 200.0             # dst_local sentinel -> one-hot row of zeros


def _prep(h, src, dst, W1, b1, W2, b2):
    """Host-side scheduling. Returns (in_maps, meta)."""
    src = np.asarray(src).astype(np.int64)
    dst = np.asarray(dst).astype(np.int64)
    h = np.asarray(h, dtype=np.float32)

    deg = np.bincount(dst, minlength=N_NODES).astype(np.float32)
    recip = (1.0 / np.maximum(deg, 1.0)).astype(np.float32)

    h16 = h.astype(BF16)
    tabA = np.vstack([h16[:SPLIT], np.zeros((1, D), BF16)])          # [32768, D]
    tabB = np.vstack([h16[SPLIT:], np.zeros((1, D), BF16)])          # [17234, D]

    gblk = dst // P                                   # global block id, 0..391
    slot = dst % P
    cls = (src >= SPLIT).astype(np.int64)

    # per-global-block class counts -> balanced assignment of blocks to
    # (core, position): sort blocks by chunk needs, deal 8 consecutive per
    # position so the per-position max over cores is tight.
    gcnt = np.bincount(gblk * 2 + cls, minlength=NCORES * NB * 2).reshape(NCORES * NB, 2)
    need = np.ceil(gcnt / P).astype(np.int64)         # [392, 2]
    blk_order = np.lexsort((need[:, 1], need[:, 0]))[::-1]
    asg = blk_order.reshape(NB, NCORES)               # asg[j, c] = global block
    # interleave big/small positions so the pipeline tail isn't a train of
    # tiny blocks (their fixed per-block latency starves the DMA stream)
    riffle = []
    lo, hi = 0, NB - 1
    while lo <= hi:
        riffle.append(lo)
        if lo != hi:
            riffle.append(hi)
        lo += 1
        hi -= 1
    asg = asg[np.array(riffle)]
    # owner/position lookup per global block
    pos_of = np.empty(NCORES * NB, np.int64)
    core_of = np.empty(NCORES * NB, np.int64)
    for j in range(NB):
        for c in range(NCORES):
            g = asg[j, c]
            pos_of[g] = j
            core_of[g] = c
    owner = core_of[gblk]
    blk = pos_of[gblk]

    # counts per (core, position, class)
    key = ((owner * NB) + blk) * 2 + cls
    cnt = np.bincount(key, minlength=NCORES * NB * 2).reshape(NCORES, NB, 2)

    # program chunk counts per block position (uniform across cores)
    kA = np.maximum(np.ceil(cnt[:, :, 0] / P).astype(np.int64).max(axis=0), 1)  # [NB]
    kB = np.ceil(cnt[:, :, 1] / P).astype(np.int64).max(axis=0)                 # [NB]
    startA = np.concatenate([[0], np.cumsum(kA)])
    startB = np.concatenate([[0], np.cumsum(kB)])
    nchA, nchB = int(startA[-1]), int(startB[-1])
    nchB = max(nchB, 1)  # keep tensor shapes nonzero if no class-B edges exist
    SA, SB = nchA * P, nchB * P

    # per-edge slot position within its (core, blk, cls) group; edges are
    # ordered by src within each group so every gather batch reads ascending
    # HBM addresses (row-buffer/bank friendliness; any order is correct)
    order = np.lexsort((src, cls, blk, owner))
    e_sorted = order
    grp = key[e_sorted]
    # position within group
    grp_change = np.concatenate([[0], np.cumsum(grp[1:] != grp[:-1])])
    first_of_grp = np.concatenate([[0], np.flatnonzero(grp[1:] != grp[:-1]) + 1])
    pos = np.arange(len(e_sorted)) - first_of_grp[grp_change]

    in_maps = []
    start = {0: startA, 1: startB}
    for c in range(NCORES):
        idxs = {0: np.full(SA, SPLIT, np.int64), 1: np.full(SB, B_ROWS, np.int64)}
        dloc = {0: np.full(SA, PAD_DLOC, np.float32), 1: np.full(SB, PAD_DLOC, np.float32)}
        rcp = {0: np.zeros(SA, np.float32), 1: np.zeros(SB, np.float32)}

        mask_c = owner[e_sorted] == c
        for k in (0, 1):
            m = mask_c & (cls[e_sorted] == k)
            es = e_sorted[m]
            ps = pos[m]
            gslot = start[k][blk[es]] * P + ps
            v = src[es] - (SPLIT if k else 0)
            idxs[k][gslot] = v
            dloc[k][gslot] = slot[es]
            rcp[k][gslot] = recip[dst[es]]

        def idx_layout(v):
            w = v.astype(np.int16).reshape(-1, 16).T        # [16, S/16]
            return np.tile(w, (8, 1)).copy()                # [128, S/16]

        def col_layout(v):
            return v.reshape(-1, P).T.copy()                # [128, nch]

        hT = np.zeros((D, NPC), np.float32)
        for j in range(NB):
            g = asg[j, c]
            lo = g * P
            hi = min(lo + P, N_NODES)
            if hi > lo:
                hT[:, j * P : j * P + (hi - lo)] = h[lo:hi].T

        # bf16 constants: W1 | W2 | iota  (dst_local values & iota are exact)
        auxW = np.concatenate(
            [
                np.asarray(W1, np.float32),
                np.asarray(W2, np.float32),
                np.tile(np.arange(P, dtype=np.float32)[None, :], (P, 1)),
            ],
            axis=1,
        ).astype(BF16)  # [128, 384]
        auxB = (np.asarray(b1, np.float32) + np.asarray(b2, np.float32))[:, None]

        in_maps.append(
            {
                "tabA": tabA,
                "tabB": tabB,
                "idxA": idx_layout(idxs[0]),
                "idxB": idx_layout(idxs[1]),
                "dlA": np.concatenate([col_layout(dloc[0]), col_layout(rcp[0])], axis=1),
                "dlB": np.concatenate([col_layout(dloc[1]), col_layout(rcp[1])], axis=1),
                "hT": hT.astype(BF16),
                "auxW": auxW,
                "auxB": auxB.astype(np.float32),
            }
        )

    meta = dict(
        kA=kA, kB=kB, startA=startA, startB=startB, nchA=nchA, nchB=nchB, asg=asg
    )
    return in_maps, meta


def _build(meta):
    import concourse.bacc as bacc
    import concourse.mybir as mybir
    import concourse.tile as tile

    kA, kB = meta["kA"], meta["kB"]
    startA, startB = meta["startA"], meta["startB"]
    nchA, nchB = meta["nchA"], meta["nchB"]
    f32 = mybir.dt.float32
    bf16 = mybir.dt.bfloat16

    nc = bacc.Bacc("TRN2", target_bir_lowering=False, debug=False, num_devices=NCORES)
    tabA = nc.declare_dram_parameter("tabA", [SPLIT + 1, D], bf16, isOutput=False)
    tabB = nc.declare_dram_parameter("tabB", [B_ROWS + 1, D], bf16, isOutput=False)
    idxA = nc.declare_dram_parameter("idxA", [P, nchA * 8], mybir.dt.int16, isOutput=False)
    idxB = nc.declare_dram_parameter("idxB", [P, nchB * 8], mybir.dt.int16, isOutput=False)
    dlA = nc.declare_dram_parameter("dlA", [P, 2 * nchA], f32, isOutput=False)
    dlB = nc.declare_dram_parameter("dlB", [P, 2 * nchB], f32, isOutput=False)
    hT = nc.declare_dram_parameter("hT", [D, NPC], bf16, isOutput=False)
    auxW = nc.declare_dram_parameter("auxW", [P, 384], bf16, isOutput=False)
    auxB = nc.declare_dram_parameter("auxB", [P, 1], f32, isOutput=False)
    outT = nc.declare_dram_parameter("outT", [D, NPC], bf16, isOutput=True)

    tabs = {0: tabA, 1: tabB}
    idx_d = {0: idxA, 1: idxB}
    dl_d = {0: dlA, 1: dlB}
    nch = {0: nchA, 1: nchB}
    bch = {0: BCH_A, 1: BCH_B}

    with tile.TileContext(nc) as tc, ExitStack() as ctx:
        consts = ctx.enter_context(tc.tile_pool(name="consts", bufs=1))
        gA_pool = ctx.enter_context(tc.tile_pool(name="gA", bufs=4))
        gB_pool = ctx.enter_context(tc.tile_pool(name="gB", bufs=4))
        oh_pool = ctx.enter_context(tc.tile_pool(name="oh", bufs=12))
        hT_pool = ctx.enter_context(tc.tile_pool(name="hTp", bufs=3))
        hN_pool = ctx.enter_context(tc.tile_pool(name="hNp", bufs=3))
        out_pool = ctx.enter_context(tc.tile_pool(name="outp", bufs=3))
        psA = ctx.enter_context(tc.tile_pool(name="psA", bufs=4, space="PSUM"))
        psO = ctx.enter_context(tc.tile_pool(name="psO", bufs=4, space="PSUM"))

        # constant loads; head slices land first so the gather/onehot pipeline
        # can start while the bulk loads behind it.
        HEAD = 2 * BCH_A
        idx_t = {}
        dl_t = {}
        for k in (0, 1):
            it = consts.tile([P, nch[k] * 8], mybir.dt.int16, name=f"idx{k}")
            h8 = min(HEAD * 8, nch[k] * 8)
            nc.sync.dma_start(it[:, :h8], idx_d[k][:, :h8])
            if nch[k] * 8 > h8:
                nc.sync.dma_start(it[:, h8:], idx_d[k][:, h8:])
            idx_t[k] = it
            dt_ = consts.tile([P, 2 * nch[k]], f32, name=f"dl{k}")
            hd = min(HEAD, nch[k])
            nc.sync.dma_start(dt_[:, :hd], dl_d[k][:, :hd])
            nc.sync.dma_start(
                dt_[:, nch[k] : nch[k] + hd], dl_d[k][:, nch[k] : nch[k] + hd]
            )
            if nch[k] > hd:
                nc.sync.dma_start(dt_[:, hd : nch[k]], dl_d[k][:, hd : nch[k]])
                nc.sync.dma_start(dt_[:, nch[k] + hd :], dl_d[k][:, nch[k] + hd :])
            dl_t[k] = dt_
        auxW_t = consts.tile([P, 384], bf16)
        nc.sync.dma_start(auxW_t[:], auxW[:])
        auxB_t = consts.tile([P, 1], f32)
        nc.sync.dma_start(auxB_t[:], auxB[:])
        w1_s = auxW_t[:, 0:128]
        w2_s = auxW_t[:, 128:256]
        iota_s = auxW_t[:, 256:384]
        bias_s = auxB_t[:, 0:1]

        g_pool = {0: gA_pool, 1: gB_pool}
        g_state = {0: (-1, None), 1: (-1, None)}

        def chunk_ap(k, j):
            """AP of gathered G chunk j of stream k (emits batch gather on demand)."""
            b, off = divmod(j, bch[k])
            cur, tile_ = g_state[k]
            if b != cur:
                nchunks = min(bch[k], nch[k] - b * bch[k])
                gt = g_pool[k].tile([P, bch[k] * D], bf16, name=f"g{k}")
                ni = nchunks * P
                nc.gpsimd.dma_gather(
                    gt[:, : nchunks * D].rearrange("p (c e) -> p c e", e=D),
                    tabs[k][:, :],
                    idx_t[k][:, b * bch[k] * 8 : b * bch[k] * 8 + (ni // 16)],
                    ni,
                    ni,
                    D,
                    single_packet=False,
                )
                g_state[k] = (b, gt)
                tile_ = gt
            return tile_[:, off * D : (off + 1) * D]

        for j in range(NB):
            agg = psA.tile([P, P], f32)
            plan = [(0, int(startA[j]) + t) for t in range(int(kA[j]))] + [
                (1, int(startB[j]) + t) for t in range(int(kB[j]))
            ]
            for i, (k, ch) in enumerate(plan):
                g_ap = chunk_ap(k, ch)
                oh = oh_pool.tile([P, P], bf16)
                nc.vector.tensor_scalar(
                    out=oh[:],
                    in0=iota_s,
                    scalar1=dl_t[k][:, ch : ch + 1],
                    scalar2=dl_t[k][:, nch[k] + ch : nch[k] + ch + 1],
                    op0=mybir.AluOpType.is_equal,
                    op1=mybir.AluOpType.mult,
                )
                nc.tensor.matmul(
                    agg[:],
                    lhsT=g_ap,
                    rhs=oh[:],
                    start=(i == 0),
                    stop=(i == len(plan) - 1),
                )

            hN_t = hN_pool.tile([P, P], bf16)
            nc.scalar.activation(hN_t[:], agg[:], mybir.ActivationFunctionType.Copy)

            if j % 2 == 0:
                hT_t2 = hT_pool.tile([P, 2 * P], bf16)
                hi = min(2 * P, NPC - j * P)
                nc.sync.dma_start(hT_t2[:, :hi], hT[:, j * P : j * P + hi])
            hT_slice = hT_t2[:, 0:P] if j % 2 == 0 else hT_t2[:, P : 2 * P]

            po = psO.tile([P, P], f32)
            nc.tensor.matmul(po[:], lhsT=w1_s, rhs=hT_slice, start=True, stop=False)
            nc.tensor.matmul(po[:], lhsT=w2_s, rhs=hN_t[:], start=False, stop=True)

            if j % 2 == 0:
                out_t2 = out_pool.tile([P, 2 * P], bf16)
            out_sl = out_t2[:, 0:P] if j % 2 == 0 else out_t2[:, P : 2 * P]
            nc.scalar.activation(
                out_sl, po[:], mybir.ActivationFunctionType.Identity, bias=bias_s
            )
            if j % 2 == 1:
                nc.sync.dma_start(outT[:, (j - 1) * P : (j + 1) * P], out_t2[:])
            elif j == NB - 1:
                nc.sync.dma_start(outT[:, j * P : (j + 1) * P], out_t2[:, 0:P])

    nc.finalize()
    return nc


def kernel(h, src, dst, W1, b1, W2, b2):
    from concourse.bass_utils import run_bass_kernel_spmd

    in_maps, meta = _prep(h, src, dst, W1, b1, W2, b2)
    nc = _build(meta)
    res = run_bass_kernel_spmd(nc, in_maps, list(range(NCORES))).results
    return _assemble(res, meta)


def _assemble(res, meta):
    asg = meta["asg"]
    out = np.zeros((NCORES * NPC, D), np.float32)
    for j in range(NB):
        for c in range(NCORES):
            g = asg[j, c]
            out[g * P : (g + 1) * P] = (
                res[c]["outT"][:, j * P : (j + 1) * P].astype(np.float32).T
            )
    return out[:N_NODES].astype(np.float32)


def _sim(h, src, dst, W1, b1, W2, b2):
    """Numpy simulation of the exact device program (bookkeeping check)."""
    in_maps, meta = _prep(h, src, dst, W1, b1, W2, b2)
    kA, kB = meta["kA"], meta["kB"]
    startA, startB = meta["startA"], meta["startB"]
    nchA, nchB = meta["nchA"], meta["nchB"]
    outs = []
    for c in range(NCORES):
        m = in_maps[c]
        tabs = {0: m["tabA"], 1: m["tabB"]}
        start = {0: startA, 1: startB}
        kS = {0: kA, 1: kB}
        nch = {0: nchA, 1: nchB}
        # un-layout idx: [128, S/16] -> first 16 rows -> slot i = [i%16, i//16]
        idxs = {}
        dloc = {}
        rcp = {}
        for k, nm_i, nm_d in ((0, "idxA", "dlA"), (1, "idxB", "dlB")):
            w = m[nm_i][:16]
            idxs[k] = w.T.reshape(-1)
            dl = m[nm_d].astype(np.float32)
            dloc[k] = dl[:, : nch[k]]
            rcp[k] = dl[:, nch[k] :]
        W1b = m["auxW"][:, 0:128].astype(np.float32)
        W2b = m["auxW"][:, 128:256].astype(np.float32)
        outT = np.zeros((D, NPC), np.float32)
        for j in range(NB):
            agg = np.zeros((P, P), np.float32)
            for k in (0, 1):
                for t in range(int(kS[k][j])):
                    ch = int(start[k][j]) + t
                    gi = idxs[k][ch * P : (ch + 1) * P].astype(np.int64)
                    G = tabs[k][gi].astype(np.float32)     # [128 edges, D]
                    O = (dloc[k][:, ch, None] == np.arange(P)[None, :]) * rcp[k][
                        :, ch, None
                    ]
                    O = O.astype(BF16).astype(np.float32)
                    agg += G.T @ O
            hTj = m["hT"][:, j * P : (j + 1) * P].astype(np.float32)
            hNj = agg.astype(BF16).astype(np.float32)
            po = W1b.T @ hTj + W2b.T @ hNj
            outT[:, j * P : (j + 1) * P] = (po + m["auxB"]).astype(BF16)
        outs.append({"outT": outT.astype(BF16)})
    return _assemble(outs, meta)


if __name__ == "__main__":
    # quick host-side validation against the reference math
    rng = np.random.default_rng(0)
    h = rng.standard_normal((N_NODES, D), dtype=np.float32)
    src = rng.integers(0, N_NODES, N_EDGES)
    dst = rng.integers(0, N_NODES, N_EDGES)
    W1 = rng.standard_normal((D, D), dtype=np.float32) * 0.1
    b1 = rng.standard_normal(D, dtype=np.float32) * 0.1
    W2 = rng.standard_normal((D, D), dtype=np.float32) * 0.1
    b2 = rng.standard_normal(D, dtype=np.float32) * 0.1

    msgs_sum = np.zeros((N_NODES, D), np.float32)
    np.add.at(msgs_sum, dst, h[src])
    deg = np.bincount(dst, minlength=N_NODES).astype(np.float32)
    hN = msgs_sum / np.maximum(deg, 1.0)[:, None]
    ref = h @ W1 + b1 + hN @ W2 + b2

    got = _sim(h, src, dst, W1, b1, W2, b2)
    err = np.linalg.norm(got - ref) / np.linalg.norm(ref)
    print("sim rel err (norm):", err)
    print("sim max abs err:", np.abs(got - ref).max())


# revision 10
# speedup vs baseline: 1.1819x; 1.0012x over previous
"""GNN message-passing (copy_u -> segment mean -> two GEMMs) on 8 trn2 NeuronCores.

Strategy (1D dst partitioning, per sharding hint):
  - Nodes are split into 392 blocks of 128; blocks are dealt to the 8 cores
    sorted by in-edge chunk count so per-position chunk counts (which the
    SPMD program must fix uniformly) stay tight. All in-edges of a node live
    on its owner core.
  - h is shipped in bf16. Per block, edges are gathered 128 at a time from
    HBM via dma_gather (256B rows). int16 gather indices only reach 32768
    rows, so h is shipped as two tables (rows [0,32767) and [32767,50000)),
    each with a trailing all-zero row used for pad slots.
  - Aggregation per 128-edge chunk: psum[feat,dst] += G_chunk.T @ O_chunk on
    the PE (bf16: 1 cycle/row vs 4 for fp32), where
    O[e,d] = (dst_local[e]==d) * recip_deg[dst_e] is built on the DVE with a
    single fused tensor_scalar (is_equal, mult) per chunk, in bf16 (2x DVE).
    This produces h_N.T (feature-major) with the mean division folded in.
  - Per block: psum_out[dout,node] = W1.T @ hT_blk + W2.T @ hNT_blk
    (weights stationary, all bf16), bias added during the ScalarE PSUM
    evacuation (activation Identity with per-partition fp32 bias). Output is
    written feature-major in bf16 and transposed back on the host.

Self-contained: only needs numpy + the concourse stack at /opt/trn_rl_repo.
"""

import sys

if "/opt/trn_rl_repo" not in sys.path:
    sys.path.insert(0, "/opt/trn_rl_repo")

import numpy as np
from contextlib import ExitStack

import ml_dtypes

BF16 = ml_dtypes.bfloat16

N_NODES = 50000
N_EDGES = 800000
D = 128
P = 128
NCORES = 8
NB = 49                      # blocks per core
NPC = NB * P                 # nodes per core (6272)
NTOT = NPC * NCORES          # padded node count (50176)
SPLIT = 32767                # table A holds h rows [0, SPLIT); idx SPLIT = zero row
B_ROWS = N_NODES - SPLIT     # 17233 data rows in table B
BCH_A = 32                   # chunks per gather batch (4096 idxs)
BCH_B = 32                   # chunks per gather batch, B stream
PAD_DLOC = 200.0             # dst_local sentinel -> one-hot row of zeros


def _prep(h, src, dst, W1, b1, W2, b2):
    """Host-side scheduling. Returns (in_maps, meta)."""
    src = np.asarray(src).astype(np.int64)
    dst = np.asarray(dst).astype(np.int64)
    h = np.asarray(h, dtype=np.float32)

    deg = np.bincount(dst, minlength=N_NODES).astype(np.float32)
    recip = (1.0 / np.maximum(deg, 1.0)).astype(np.float32)

    h16 = h.astype(BF16)
    tabA = np.vstack([h16[:SPLIT], np.zeros((1, D), BF16)])          # [32768, D]
    tabB = np.vstack([h16[SPLIT:], np.zeros((1, D), BF16)])          # [17234, D]

    gblk = dst // P                                   # global block id, 0..391
    slot = dst % P
    cls = (src >= SPLIT).astype(np.int64)

    # per-global-block class counts -> balanced assignment of blocks to
    # (core, position): sort blocks by chunk needs, deal 8 consecutive per
    # position so the per-position max over cores is tight.
    gcnt = np.bincount(gblk * 2 + cls, minlength=NCORES * NB * 2).reshape(NCORES * NB, 2)
    need = np.ceil(gcnt / P).astype(np.int64)         # [392, 2]
    blk_order = np.lexsort((need[:, 1], need[:, 0]))[::-1]
    asg = blk_order.reshape(NB, NCORES)               # asg[j, c] = global block
    # interleave big/small positions so the pipeline tail isn't a train of
    # tiny blocks (their fixed per-block latency starves the DMA stream)
    riffle = []
    lo, hi = 0, NB - 1
    while lo <= hi:
        riffle.append(lo)
        if lo != hi:
            riffle.append(hi)
        lo += 1
        hi -= 1
    asg = asg[np.array(riffle)]
    # owner/position lookup per global block
    pos_of = np.empty(NCORES * NB, np.int64)
    core_of = np.empty(NCORES * NB, np.int64)
    for j in range(NB):
        for c in range(NCORES):
            g = asg[j, c]
            pos_of[g] = j
            core_of[g] = c
    owner = core_of[gblk]
    blk = pos_of[gblk]

    # counts per (core, position, class)
    key = ((owner * NB) + blk) * 2 + cls
    cnt = np.bincount(key, minlength=NCORES * NB * 2).reshape(NCORES, NB, 2)

    # program chunk counts per block position (uniform across cores)
    kA = np.maximum(np.ceil(cnt[:, :, 0] / P).astype(np.int64).max(axis=0), 1)  # [NB]
    kB = np.ceil(cnt[:, :, 1] / P).astype(np.int64).max(axis=0)                 # [NB]
    startA = np.concatenate([[0], np.cumsum(kA)])
    startB = np.concatenate([[0], np.cumsum(kB)])
    nchA, nchB = int(startA[-1]), int(startB[-1])
    nchB = max(nchB, 1)  # keep tensor shapes nonzero if no class-B edges exist
    SA, SB = nchA * P, nchB * P

    # per-edge slot position within its (core, blk, cls) group; edges are
    # ordered by src within each group so every gather batch reads ascending
    # HBM addresses (row-buffer/bank friendliness; any order is correct)
    order = np.lexsort((src, cls, blk, owner))
    e_sorted = order
    grp = key[e_sorted]
    # position within group
    grp_change = np.concatenate([[0], np.cumsum(grp[1:] != grp[:-1])])
    first_of_grp = np.concatenate([[0], np.flatnonzero(grp[1:] != grp[:-1]) + 1])
    pos = np.arange(len(e_sorted)) - first_of_grp[grp_change]

    in_maps = []
    start = {0: startA, 1: startB}
    for c in range(NCORES):
        idxs = {0: np.full(SA, SPLIT, np.int64), 1: np.full(SB, B_ROWS, np.int64)}
        dloc = {0: np.full(SA, PAD_DLOC, np.float32), 1: np.full(SB, PAD_DLOC, np.float32)}
        rcp = {0: np.zeros(SA, np.float32), 1: np.zeros(SB, np.float32)}

        mask_c = owner[e_sorted] == c
        for k in (0, 1):
            m = mask_c & (cls[e_sorted] == k)
            es = e_sorted[m]
            ps = pos[m]
            gslot = start[k][blk[es]] * P + ps
            v = src[es] - (SPLIT if k else 0)
            idxs[k][gslot] = v
            dloc[k][gslot] = slot[es]
            rcp[k][gslot] = recip[dst[es]]

        def idx_layout(v):
            w = v.astype(np.int16).reshape(-1, 16).T        # [16, S/16]
            return np.tile(w, (8, 1)).copy()                # [128, S/16]

        def col_layout(v):
            return v.reshape(-1, P).T.copy()                # [128, nch]

        hT = np.zeros((D, NPC), np.float32)
        for j in range(NB):
            g = asg[j, c]
            lo = g * P
            hi = min(lo + P, N_NODES)
            if hi > lo:
                hT[:, j * P : j * P + (hi - lo)] = h[lo:hi].T

        # bf16 constants: W1 | W2 | iota  (dst_local values & iota are exact)
        auxW = np.concatenate(
            [
                np.asarray(W1, np.float32),
                np.asarray(W2, np.float32),
                np.tile(np.arange(P, dtype=np.float32)[None, :], (P, 1)),
            ],
            axis=1,
        ).astype(BF16)  # [128, 384]
        auxB = (np.asarray(b1, np.float32) + np.asarray(b2, np.float32))[:, None]

        in_maps.append(
            {
                "tabA": tabA,
                "tabB": tabB,
                "idxA": idx_layout(idxs[0]),
                "idxB": idx_layout(idxs[1]),
                "dlA": np.concatenate([col_layout(dloc[0]), col_layout(rcp[0])], axis=1),
                "dlB": np.concatenate([col_layout(dloc[1]), col_layout(rcp[1])], axis=1),
                "hT": hT.astype(BF16),
                "auxW": auxW,
                "auxB": auxB.astype(np.float32),
            }
        )

    meta = dict(
        kA=kA, kB=kB, startA=startA, startB=startB, nchA=nchA, nchB=nchB, asg=asg
    )
    return in_maps, meta


def _build(meta):
    import concourse.bacc as bacc
    import concourse.mybir as mybir
    import concourse.tile as tile

    kA, kB = meta["kA"], meta["kB"]
    startA, startB = meta["startA"], meta["startB"]
    nchA, nchB = meta["nchA"], meta["nchB"]
    f32 = mybir.dt.float32
    bf16 = mybir.dt.bfloat16

    nc = bacc.Bacc("TRN2", target_bir_lowering=False, debug=False, num_devices=NCORES)
    tabA = nc.declare_dram_parameter("tabA", [SPLIT + 1, D], bf16, isOutput=False)
    tabB = nc.declare_dram_parameter("tabB", [B_ROWS + 1, D], bf16, isOutput=False)
    idxA = nc.declare_dram_parameter("idxA", [P, nchA * 8], mybir.dt.int16, isOutput=False)
    idxB = nc.declare_dram_parameter("idxB", [P, nchB * 8], mybir.dt.int16, isOutput=False)
    dlA = nc.declare_dram_parameter("dlA", [P, 2 * nchA], f32, isOutput=False)
    dlB = nc.declare_dram_parameter("dlB", [P, 2 * nchB], f32, isOutput=False)
    hT = nc.declare_dram_parameter("hT", [D, NPC], bf16, isOutput=False)
    auxW = nc.declare_dram_parameter("auxW", [P, 384], bf16, isOutput=False)
    auxB = nc.declare_dram_parameter("auxB", [P, 1], f32, isOutput=False)
    outT = nc.declare_dram_parameter("outT", [D, NPC], bf16, isOutput=True)

    tabs = {0: tabA, 1: tabB}
    idx_d = {0: idxA, 1: idxB}
    dl_d = {0: dlA, 1: dlB}
    nch = {0: nchA, 1: nchB}
    bch = {0: BCH_A, 1: BCH_B}

    with tile.TileContext(nc) as tc, ExitStack() as ctx:
        consts = ctx.enter_context(tc.tile_pool(name="consts", bufs=1))
        gA_pool = ctx.enter_context(tc.tile_pool(name="gA", bufs=5))
        gB_pool = ctx.enter_context(tc.tile_pool(name="gB", bufs=4))
        oh_pool = ctx.enter_context(tc.tile_pool(name="oh", bufs=48))
        hT_pool = ctx.enter_context(tc.tile_pool(name="hTp", bufs=3))
        hN_pool = ctx.enter_context(tc.tile_pool(name="hNp", bufs=4))
        out_pool = ctx.enter_context(tc.tile_pool(name="outp", bufs=4))
        psA = ctx.enter_context(tc.tile_pool(name="psA", bufs=5, space="PSUM"))
        psO = ctx.enter_context(tc.tile_pool(name="psO", bufs=3, space="PSUM"))

        # constant loads; head slices land first so the gather/onehot pipeline
        # can start while the bulk loads behind it.
        HEAD = 2 * BCH_A
        idx_t = {}
        dl_t = {}
        for k in (0, 1):
            it = consts.tile([P, nch[k] * 8], mybir.dt.int16, name=f"idx{k}")
            h8 = min(HEAD * 8, nch[k] * 8)
            nc.sync.dma_start(it[:, :h8], idx_d[k][:, :h8])
            if nch[k] * 8 > h8:
                nc.sync.dma_start(it[:, h8:], idx_d[k][:, h8:])
            idx_t[k] = it
            dt_ = consts.tile([P, 2 * nch[k]], f32, name=f"dl{k}")
            hd = min(HEAD, nch[k])
            nc.sync.dma_start(dt_[:, :hd], dl_d[k][:, :hd])
            nc.sync.dma_start(
                dt_[:, nch[k] : nch[k] + hd], dl_d[k][:, nch[k] : nch[k] + hd]
            )
            if nch[k] > hd:
                nc.sync.dma_start(dt_[:, hd : nch[k]], dl_d[k][:, hd : nch[k]])
                nc.sync.dma_start(dt_[:, nch[k] + hd :], dl_d[k][:, nch[k] + hd :])
            dl_t[k] = dt_
        auxW_t = consts.tile([P, 384], bf16)
        nc.sync.dma_start(auxW_t[:], auxW[:])
        auxB_t = consts.tile([P, 1], f32)
        nc.sync.dma_start(auxB_t[:], auxB[:])
        w1_s = auxW_t[:, 0:128]
        w2_s = auxW_t[:, 128:256]
        iota_s = auxW_t[:, 256:384]
        bias_s = auxB_t[:, 0:1]

        g_pool = {0: gA_pool, 1: gB_pool}
        g_state = {0: (-1, None), 1: (-1, None)}

        def chunk_ap(k, j):
            """AP of gathered G chunk j of stream k (emits batch gather on demand)."""
            b, off = divmod(j, bch[k])
            cur, tile_ = g_state[k]
            if b != cur:
                nchunks = min(bch[k], nch[k] - b * bch[k])
                gt = g_pool[k].tile([P, bch[k] * D], bf16, name=f"g{k}")
                ni = nchunks * P
                nc.gpsimd.dma_gather(
                    gt[:, : nchunks * D].rearrange("p (c e) -> p c e", e=D),
                    tabs[k][:, :],
                    idx_t[k][:, b * bch[k] * 8 : b * bch[k] * 8 + (ni // 16)],
                    ni,
                    ni,
                    D,
                    single_packet=False,
                )
                g_state[k] = (b, gt)
                tile_ = gt
            return tile_[:, off * D : (off + 1) * D]

        for j in range(NB):
            agg = psA.tile([P, P], f32)
            plan = [(0, int(startA[j]) + t) for t in range(int(kA[j]))] + [
                (1, int(startB[j]) + t) for t in range(int(kB[j]))
            ]
            for i, (k, ch) in enumerate(plan):
                g_ap = chunk_ap(k, ch)
                oh = oh_pool.tile([P, P], bf16)
                nc.vector.tensor_scalar(
                    out=oh[:],
                    in0=iota_s,
                    scalar1=dl_t[k][:, ch : ch + 1],
                    scalar2=dl_t[k][:, nch[k] + ch : nch[k] + ch + 1],
                    op0=mybir.AluOpType.is_equal,
                    op1=mybir.AluOpType.mult,
                )
                nc.tensor.matmul(
                    agg[:],
                    lhsT=g_ap,
                    rhs=oh[:],
                    start=(i == 0),
                    stop=(i == len(plan) - 1),
                )

            hN_t = hN_pool.tile([P, P], bf16)
            nc.scalar.activation(hN_t[:], agg[:], mybir.ActivationFunctionType.Copy)

            if j % 2 == 0:
                hT_t2 = hT_pool.tile([P, 2 * P], bf16)
                hi = min(2 * P, NPC - j * P)
                nc.sync.dma_start(hT_t2[:, :hi], hT[:, j * P : j * P + hi])
            hT_slice = hT_t2[:, 0:P] if j % 2 == 0 else hT_t2[:, P : 2 * P]

            po = psO.tile([P, P], f32)
            nc.tensor.matmul(po[:], lhsT=w1_s, rhs=hT_slice, start=True, stop=False)
            nc.tensor.matmul(po[:], lhsT=w2_s, rhs=hN_t[:], start=False, stop=True)

            if j % 2 == 0:
                out_t2 = out_pool.tile([P, 2 * P], bf16)
            out_sl = out_t2[:, 0:P] if j % 2 == 0 else out_t2[:, P : 2 * P]
            nc.scalar.activation(
                out_sl, po[:], mybir.ActivationFunctionType.Identity, bias=bias_s
            )
            if j % 2 == 1:
                nc.sync.dma_start(outT[:, (j - 1) * P : (j + 1) * P], out_t2[:])
            elif j == NB - 1:
                nc.sync.dma_start(outT[:, j * P : (j + 1) * P], out_t2[:, 0:P])

    nc.finalize()
    return nc


def kernel(h, src, dst, W1, b1, W2, b2):
    from concourse.bass_utils import run_bass_kernel_spmd

    in_maps, meta = _prep(h, src, dst, W1, b1, W2, b2)
    nc = _build(meta)
    res = run_bass_kernel_spmd(nc, in_maps, list(range(NCORES))).results
    return _assemble(res, meta)


def _assemble(res, meta):
    asg = meta["asg"]
    out = np.zeros((NCORES * NPC, D), np.float32)
    for j in range(NB):
        for c in range(NCORES):
            g = asg[j, c]
            out[g * P : (g + 1) * P] = (
                res[c]["outT"][:, j * P : (j + 1) * P].astype(np.float32).T
            )
    return out[:N_NODES].astype(np.float32)


def _sim(h, src, dst, W1, b1, W2, b2):
    """Numpy simulation of the exact device program (bookkeeping check)."""
    in_maps, meta = _prep(h, src, dst, W1, b1, W2, b2)
    kA, kB = meta["kA"], meta["kB"]
    startA, startB = meta["startA"], meta["startB"]
    nchA, nchB = meta["nchA"], meta["nchB"]
    outs = []
    for c in range(NCORES):
        m = in_maps[c]
        tabs = {0: m["tabA"], 1: m["tabB"]}
        start = {0: startA, 1: startB}
        kS = {0: kA, 1: kB}
        nch = {0: nchA, 1: nchB}
        # un-layout idx: [128, S/16] -> first 16 rows -> slot i = [i%16, i//16]
        idxs = {}
        dloc = {}
        rcp = {}
        for k, nm_i, nm_d in ((0, "idxA", "dlA"), (1, "idxB", "dlB")):
            w = m[nm_i][:16]
            idxs[k] = w.T.reshape(-1)
            dl = m[nm_d].astype(np.float32)
            dloc[k] = dl[:, : nch[k]]
            rcp[k] = dl[:, nch[k] :]
        W1b = m["auxW"][:, 0:128].astype(np.float32)
        W2b = m["auxW"][:, 128:256].astype(np.float32)
        outT = np.zeros((D, NPC), np.float32)
        for j in range(NB):
            agg = np.zeros((P, P), np.float32)
            for k in (0, 1):
                for t in range(int(kS[k][j])):
                    ch = int(start[k][j]) + t
                    gi = idxs[k][ch * P : (ch + 1) * P].astype(np.int64)
                    G = tabs[k][gi].astype(np.float32)     # [128 edges, D]
                    O = (dloc[k][:, ch, None] == np.arange(P)[None, :]) * rcp[k][
                        :, ch, None
                    ]
                    O = O.astype(BF16).astype(np.float32)
                    agg += G.T @ O
            hTj = m["hT"][:, j * P : (j + 1) * P].astype(np.float32)
            hNj = agg.astype(BF16).astype(np.float32)
            po = W1b.T @ hTj + W2b.T @ hNj
            outT[:, j * P : (j + 1) * P] = (po + m["auxB"]).astype(BF16)
        outs.append({"outT": outT.astype(BF16)})
    return _assemble(outs, meta)


if __name__ == "__main__":
    # quick host-side validation against the reference math
    rng = np.random.default_rng(0)
    h = rng.standard_normal((N_NODES, D), dtype=np.float32)
    src = rng.integers(0, N_NODES, N_EDGES)
    dst = rng.integers(0, N_NODES, N_EDGES)
    W1 = rng.standard_normal((D, D), dtype=np.float32) * 0.1
    b1 = rng.standard_normal(D, dtype=np.float32) * 0.1
    W2 = rng.standard_normal((D, D), dtype=np.float32) * 0.1
    b2 = rng.standard_normal(D, dtype=np.float32) * 0.1

    msgs_sum = np.zeros((N_NODES, D), np.float32)
    np.add.at(msgs_sum, dst, h[src])
    deg = np.bincount(dst, minlength=N_NODES).astype(np.float32)
    hN = msgs_sum / np.maximum(deg, 1.0)[:, None]
    ref = h @ W1 + b1 + hN @ W2 + b2

    got = _sim(h, src, dst, W1, b1, W2, b2)
    err = np.linalg.norm(got - ref) / np.linalg.norm(ref)
    print("sim rel err (norm):", err)
    print("sim max abs err:", np.abs(got - ref).max())


# revision 11
# speedup vs baseline: 1.1875x; 1.0047x over previous
"""GNN message-passing (copy_u -> segment mean -> two GEMMs) on 8 trn2 NeuronCores.

Strategy (1D dst partitioning, per sharding hint):
  - Nodes are split into 392 blocks of 128; blocks are dealt to the 8 cores
    sorted by in-edge chunk count so per-position chunk counts (which the
    SPMD program must fix uniformly) stay tight. All in-edges of a node live
    on its owner core.
  - h is shipped in bf16. Per block, edges are gathered 128 at a time from
    HBM via dma_gather (256B rows). int16 gather indices only reach 32768
    rows, so h is shipped as two tables (rows [0,32767) and [32767,50000)),
    each with a trailing all-zero row used for pad slots.
  - Aggregation per 128-edge chunk: psum[feat,dst] += G_chunk.T @ O_chunk on
    the PE (bf16), with O[e,d] = (dst_local[e]==d) a pure 0/1 one-hot. One
    DVE tensor_tensor(is_equal) builds one-hots for OHB chunks at once in an
    interleaved layout oh[e, d*OHB+c] (keeps every operand's innermost dim
    stride-1 so the DVE 2x mode applies); each matmul reads its chunk as a
    stride-OHB view. Batching cuts DVE instruction count ~8x, which removes
    the per-chunk semaphore round-trips that paced the pipeline drain.
  - The mean division: hN = psum * recip[dst] applied during the DVE PSUM
    evacuation as a tensor_tensor multiply against a [128, NPC] recip-
    broadcast tile built once on GPSIMD (partition_broadcast of a host
    [1, NPC] row).
  - Per block: psum_out[dout,node] = W1.T @ hT_blk + W2.T @ hNT_blk
    (weights stationary, bf16), bias added during the ScalarE PSUM
    evacuation (activation Identity with per-partition fp32 bias). Output is
    written feature-major in bf16, two blocks per DMA (512B descriptors),
    and transposed back on the host.

Self-contained: only needs numpy + the concourse stack at /opt/trn_rl_repo.
"""

import sys

if "/opt/trn_rl_repo" not in sys.path:
    sys.path.insert(0, "/opt/trn_rl_repo")

import numpy as np
from contextlib import ExitStack

import ml_dtypes

BF16 = ml_dtypes.bfloat16

N_NODES = 50000
N_EDGES = 800000
D = 128
P = 128
NCORES = 8
NB = 49                      # blocks per core
NPC = NB * P                 # nodes per core (6272)
SPLIT = 32767                # table A holds h rows [0, SPLIT); idx SPLIT = zero row
B_ROWS = N_NODES - SPLIT     # 17233 data rows in table B
BCH_A = 32                   # chunks per gather batch (4096 idxs)
BCH_B = 32                   # chunks per gather batch, B stream
OHB = 8                      # chunks per one-hot build
PAD_DLOC = 200.0             # dst_local sentinel -> one-hot column of zeros


def _prep(h, src, dst, W1, b1, W2, b2):
    """Host-side scheduling. Returns (in_maps, meta)."""
    src = np.asarray(src).astype(np.int64)
    dst = np.asarray(dst).astype(np.int64)
    h = np.asarray(h, dtype=np.float32)

    deg = np.bincount(dst, minlength=N_NODES).astype(np.float32)
    recip = (1.0 / np.maximum(deg, 1.0)).astype(np.float32)

    h16 = h.astype(BF16)
    tabA = np.vstack([h16[:SPLIT], np.zeros((1, D), BF16)])          # [32768, D]
    tabB = np.vstack([h16[SPLIT:], np.zeros((1, D), BF16)])          # [17234, D]

    gblk = dst // P                                   # global block id, 0..391
    slot = dst % P
    cls = (src >= SPLIT).astype(np.int64)

    # per-global-block class counts -> balanced assignment of blocks to
    # (core, position): sort blocks by chunk needs, deal 8 consecutive per
    # position so the per-position max over cores is tight.
    gcnt = np.bincount(gblk * 2 + cls, minlength=NCORES * NB * 2).reshape(NCORES * NB, 2)
    need = np.ceil(gcnt / P).astype(np.int64)         # [392, 2]
    blk_order = np.lexsort((need[:, 1], need[:, 0]))[::-1]
    asg = blk_order.reshape(NB, NCORES)               # asg[j, c] = global block
    # interleave big/small positions so the pipeline tail isn't a train of
    # tiny blocks (their fixed per-block latency starves the DMA stream)
    riffle = []
    lo, hi = 0, NB - 1
    while lo <= hi:
        riffle.append(lo)
        if lo != hi:
            riffle.append(hi)
        lo += 1
        hi -= 1
    asg = asg[np.array(riffle)]
    # owner/position lookup per global block
    pos_of = np.empty(NCORES * NB, np.int64)
    core_of = np.empty(NCORES * NB, np.int64)
    for j in range(NB):
        for c in range(NCORES):
            g = asg[j, c]
            pos_of[g] = j
            core_of[g] = c
    owner = core_of[gblk]
    blk = pos_of[gblk]

    # counts per (core, position, class)
    key = ((owner * NB) + blk) * 2 + cls
    cnt = np.bincount(key, minlength=NCORES * NB * 2).reshape(NCORES, NB, 2)

    # program chunk counts per block position (uniform across cores)
    kA = np.maximum(np.ceil(cnt[:, :, 0] / P).astype(np.int64).max(axis=0), 1)  # [NB]
    kB = np.ceil(cnt[:, :, 1] / P).astype(np.int64).max(axis=0)                 # [NB]
    startA = np.concatenate([[0], np.cumsum(kA)])
    startB = np.concatenate([[0], np.cumsum(kB)])
    nchA, nchB = int(startA[-1]), int(startB[-1])
    nchB = max(nchB, 1)  # keep tensor shapes nonzero if no class-B edges exist
    SA, SB = nchA * P, nchB * P
    # dl tensors padded to a multiple of OHB so batched one-hot builds stay
    # in bounds (pad columns hold PAD_DLOC -> all-zero one-hots, never used)
    nchAp = -(-nchA // OHB) * OHB
    nchBp = -(-nchB // OHB) * OHB

    # per-edge slot position within its (core, blk, cls) group; edges are
    # ordered by src within each group so every gather batch reads ascending
    # HBM addresses (row-buffer/bank friendliness; any order is correct)
    order = np.lexsort((src, cls, blk, owner))
    e_sorted = order
    grp = key[e_sorted]
    # position within group
    grp_change = np.concatenate([[0], np.cumsum(grp[1:] != grp[:-1])])
    first_of_grp = np.concatenate([[0], np.flatnonzero(grp[1:] != grp[:-1]) + 1])
    pos = np.arange(len(e_sorted)) - first_of_grp[grp_change]

    # interleaved iota constant: value d at column d*OHB + c
    iota_int = np.repeat(np.arange(P, dtype=np.float32), OHB)[None, :]
    iota_int = np.tile(iota_int, (P, 1)).astype(BF16)               # [128, 128*OHB]

    in_maps = []
    start = {0: startA, 1: startB}
    for c in range(NCORES):
        idxs = {0: np.full(SA, SPLIT, np.int64), 1: np.full(SB, B_ROWS, np.int64)}
        dloc = {
            0: np.full(nchAp * P, PAD_DLOC, np.float32),
            1: np.full(nchBp * P, PAD_DLOC, np.float32),
        }

        mask_c = owner[e_sorted] == c
        for k in (0, 1):
            m = mask_c & (cls[e_sorted] == k)
            es = e_sorted[m]
            ps = pos[m]
            gslot = start[k][blk[es]] * P + ps
            v = src[es] - (SPLIT if k else 0)
            idxs[k][gslot] = v
            dloc[k][gslot] = slot[es]

        def idx_layout(v):
            w = v.astype(np.int16).reshape(-1, 16).T        # [16, S/16]
            return np.tile(w, (8, 1)).copy()                # [128, S/16]

        def col_layout(v):
            return v.reshape(-1, P).T.copy()                # [128, nch]

        hT = np.zeros((D, NPC), np.float32)
        rcp = np.zeros((1, NPC), np.float32)
        for j in range(NB):
            g = asg[j, c]
            lo = g * P
            hi = min(lo + P, N_NODES)
            if hi > lo:
                hT[:, j * P : j * P + (hi - lo)] = h[lo:hi].T
                rcp[0, j * P : j * P + (hi - lo)] = recip[lo:hi]

        auxW = np.concatenate(
            [np.asarray(W1, np.float32), np.asarray(W2, np.float32)], axis=1
        ).astype(BF16)  # [128, 256]
        auxB = (np.asarray(b1, np.float32) + np.asarray(b2, np.float32))[:, None]

        in_maps.append(
            {
                "tabA": tabA,
                "tabB": tabB,
                "idxA": idx_layout(idxs[0]),
                "idxB": idx_layout(idxs[1]),
                "dlA": col_layout(dloc[0]).astype(BF16),
                "dlB": col_layout(dloc[1]).astype(BF16),
                "hT": hT.astype(BF16),
                "rcp": rcp.astype(BF16),
                "ioI": iota_int,
                "auxW": auxW,
                "auxB": auxB.astype(np.float32),
            }
        )

    meta = dict(
        kA=kA,
        kB=kB,
        startA=startA,
        startB=startB,
        nchA=nchA,
        nchB=nchB,
        nchAp=nchAp,
        nchBp=nchBp,
        asg=asg,
    )
    return in_maps, meta


def _build(meta):
    import concourse.bacc as bacc
    import concourse.mybir as mybir
    import concourse.tile as tile

    kA, kB = meta["kA"], meta["kB"]
    startA, startB = meta["startA"], meta["startB"]
    nchA, nchB = meta["nchA"], meta["nchB"]
    nchAp, nchBp = meta["nchAp"], meta["nchBp"]
    f32 = mybir.dt.float32
    bf16 = mybir.dt.bfloat16

    nc = bacc.Bacc("TRN2", target_bir_lowering=False, debug=False, num_devices=NCORES)
    tabA = nc.declare_dram_parameter("tabA", [SPLIT + 1, D], bf16, isOutput=False)
    tabB = nc.declare_dram_parameter("tabB", [B_ROWS + 1, D], bf16, isOutput=False)
    idxA = nc.declare_dram_parameter("idxA", [P, nchA * 8], mybir.dt.int16, isOutput=False)
    idxB = nc.declare_dram_parameter("idxB", [P, nchB * 8], mybir.dt.int16, isOutput=False)
    dlA = nc.declare_dram_parameter("dlA", [P, nchAp], bf16, isOutput=False)
    dlB = nc.declare_dram_parameter("dlB", [P, nchBp], bf16, isOutput=False)
    hT = nc.declare_dram_parameter("hT", [D, NPC], bf16, isOutput=False)
    rcp = nc.declare_dram_parameter("rcp", [1, NPC], bf16, isOutput=False)
    ioI = nc.declare_dram_parameter("ioI", [P, P * OHB], bf16, isOutput=False)
    auxW = nc.declare_dram_parameter("auxW", [P, 256], bf16, isOutput=False)
    auxB = nc.declare_dram_parameter("auxB", [P, 1], f32, isOutput=False)
    outT = nc.declare_dram_parameter("outT", [D, NPC], bf16, isOutput=True)

    tabs = {0: tabA, 1: tabB}
    idx_d = {0: idxA, 1: idxB}
    dl_d = {0: dlA, 1: dlB}
    nch = {0: nchA, 1: nchB}
    nchp = {0: nchAp, 1: nchBp}
    bch = {0: BCH_A, 1: BCH_B}

    with tile.TileContext(nc) as tc, ExitStack() as ctx:
        consts = ctx.enter_context(tc.tile_pool(name="consts", bufs=1))
        gA_pool = ctx.enter_context(tc.tile_pool(name="gA", bufs=5))
        gB_pool = ctx.enter_context(tc.tile_pool(name="gB", bufs=4))
        ohA_pool = ctx.enter_context(tc.tile_pool(name="ohA", bufs=10))
        ohB_pool = ctx.enter_context(tc.tile_pool(name="ohB", bufs=6))
        hT_pool = ctx.enter_context(tc.tile_pool(name="hTp", bufs=3))
        hN_pool = ctx.enter_context(tc.tile_pool(name="hNp", bufs=4))
        out_pool = ctx.enter_context(tc.tile_pool(name="outp", bufs=4))
        psA = ctx.enter_context(tc.tile_pool(name="psA", bufs=5, space="PSUM"))
        psO = ctx.enter_context(tc.tile_pool(name="psO", bufs=3, space="PSUM"))

        # constant loads; head slices land first so the gather/onehot pipeline
        # can start while the bulk loads behind it.
        HEAD = 2 * BCH_A
        idx_t = {}
        dl_t = {}
        for k in (0, 1):
            it = consts.tile([P, nch[k] * 8], mybir.dt.int16, name=f"idx{k}")
            h8 = min(HEAD * 8, nch[k] * 8)
            nc.sync.dma_start(it[:, :h8], idx_d[k][:, :h8])
            if nch[k] * 8 > h8:
                nc.sync.dma_start(it[:, h8:], idx_d[k][:, h8:])
            idx_t[k] = it
            dt_ = consts.tile([P, nchp[k]], bf16, name=f"dl{k}")
            hd = min(HEAD, nchp[k])
            nc.sync.dma_start(dt_[:, :hd], dl_d[k][:, :hd])
            if nchp[k] > hd:
                nc.sync.dma_start(dt_[:, hd:], dl_d[k][:, hd:])
            dl_t[k] = dt_
        auxW_t = consts.tile([P, 256], bf16)
        nc.sync.dma_start(auxW_t[:], auxW[:])
        auxB_t = consts.tile([P, 1], f32)
        nc.sync.dma_start(auxB_t[:], auxB[:])
        ioI_t = consts.tile([P, P * OHB], bf16)
        nc.sync.dma_start(ioI_t[:], ioI[:])
        rcp_t = consts.tile([1, NPC], bf16)
        nc.sync.dma_start(rcp_t[:], rcp[:])
        w1_s = auxW_t[:, 0:128]
        w2_s = auxW_t[:, 128:256]
        bias_s = auxB_t[:, 0:1]

        # recip broadcast across partitions, built once on GPSIMD
        rb_t = consts.tile([P, NPC], bf16)
        nc.gpsimd.partition_broadcast(rb_t[:], rcp_t[:])

        g_pool = {0: gA_pool, 1: gB_pool}
        oh_pool = {0: ohA_pool, 1: ohB_pool}
        g_state = {0: (-1, None), 1: (-1, None)}
        oh_state = {0: (-1, None), 1: (-1, None)}

        def chunk_ap(k, j):
            """AP of gathered G chunk j of stream k (emits batch gather on demand)."""
            b, off = divmod(j, bch[k])
            cur, tile_ = g_state[k]
            if b != cur:
                nchunks = min(bch[k], nch[k] - b * bch[k])
                gt = g_pool[k].tile([P, bch[k] * D], bf16, name=f"g{k}")
                ni = nchunks * P
                nc.gpsimd.dma_gather(
                    gt[:, : nchunks * D].rearrange("p (c e) -> p c e", e=D),
                    tabs[k][:, :],
                    idx_t[k][:, b * bch[k] * 8 : b * bch[k] * 8 + (ni // 16)],
                    ni,
                    ni,
                    D,
                    single_packet=False,
                )
                g_state[k] = (b, gt)
                tile_ = gt
            return tile_[:, off * D : (off + 1) * D]

        def oh_ap(k, j):
            """AP of the one-hot for chunk j of stream k ([e, d] stride-OHB view;
            emits the batched is_equal build on demand)."""
            b, ci = divmod(j, OHB)
            cur, tile_ = oh_state[k]
            if b != cur:
                ot = oh_pool[k].tile([P, P * OHB], bf16, name=f"oh{k}")
                in1 = (
                    dl_t[k][:, b * OHB : (b + 1) * OHB]
                    .rearrange("p (x c) -> p x c", x=1)
                    .broadcast_to([P, P, OHB])
                )
                nc.vector.tensor_tensor(
                    out=ot[:].rearrange("p (d c) -> p d c", c=OHB),
                    in0=ioI_t[:].rearrange("p (d c) -> p d c", c=OHB),
                    in1=in1,
                    op=mybir.AluOpType.is_equal,
                )
                oh_state[k] = (b, ot)
                tile_ = ot
            return tile_[:].rearrange("p (d c) -> p c d", c=OHB)[:, ci, :]

        for j in range(NB):
            agg = psA.tile([P, P], f32)
            plan = [(0, int(startA[j]) + t) for t in range(int(kA[j]))] + [
                (1, int(startB[j]) + t) for t in range(int(kB[j]))
            ]
            for i, (k, ch) in enumerate(plan):
                g_ap = chunk_ap(k, ch)
                nc.tensor.matmul(
                    agg[:],
                    lhsT=g_ap,
                    rhs=oh_ap(k, ch),
                    start=(i == 0),
                    stop=(i == len(plan) - 1),
                )

            # evacuate with the mean division folded in: hN = agg * recip[dst]
            hN_t = hN_pool.tile([P, P], bf16)
            nc.vector.tensor_tensor(
                out=hN_t[:],
                in0=agg[:],
                in1=rb_t[:, j * P : (j + 1) * P],
                op=mybir.AluOpType.mult,
            )

            if j % 2 == 0:
                hT_t2 = hT_pool.tile([P, 2 * P], bf16)
                hi = min(2 * P, NPC - j * P)
                nc.sync.dma_start(hT_t2[:, :hi], hT[:, j * P : j * P + hi])
            hT_slice = hT_t2[:, 0:P] if j % 2 == 0 else hT_t2[:, P : 2 * P]

            po = psO.tile([P, P], f32)
            nc.tensor.matmul(po[:], lhsT=w1_s, rhs=hT_slice, start=True, stop=False)
            nc.tensor.matmul(po[:], lhsT=w2_s, rhs=hN_t[:], start=False, stop=True)

            if j % 2 == 0:
                out_t2 = out_pool.tile([P, 2 * P], bf16)
            out_sl = out_t2[:, 0:P] if j % 2 == 0 else out_t2[:, P : 2 * P]
            nc.scalar.activation(
                out_sl, po[:], mybir.ActivationFunctionType.Identity, bias=bias_s
            )
            if j % 2 == 1:
                nc.sync.dma_start(outT[:, (j - 1) * P : (j + 1) * P], out_t2[:])
            elif j == NB - 1:
                nc.sync.dma_start(outT[:, j * P : (j + 1) * P], out_t2[:, 0:P])

    nc.finalize()
    return nc


def kernel(h, src, dst, W1, b1, W2, b2):
    from concourse.bass_utils import run_bass_kernel_spmd

    in_maps, meta = _prep(h, src, dst, W1, b1, W2, b2)
    nc = _build(meta)
    res = run_bass_kernel_spmd(nc, in_maps, list(range(NCORES))).results
    return _assemble(res, meta)


def _assemble(res, meta):
    asg = meta["asg"]
    out = np.zeros((NCORES * NPC, D), np.float32)
    for j in range(NB):
        for c in range(NCORES):
            g = asg[j, c]
            out[g * P : (g + 1) * P] = (
                res[c]["outT"][:, j * P : (j + 1) * P].astype(np.float32).T
            )
    return out[:N_NODES].astype(np.float32)


def _sim(h, src, dst, W1, b1, W2, b2):
    """Numpy simulation of the exact device program (bookkeeping check)."""
    in_maps, meta = _prep(h, src, dst, W1, b1, W2, b2)
    kA, kB = meta["kA"], meta["kB"]
    startA, startB = meta["startA"], meta["startB"]
    outs = []
    for c in range(NCORES):
        m = in_maps[c]
        tabs = {0: m["tabA"], 1: m["tabB"]}
        start = {0: startA, 1: startB}
        kS = {0: kA, 1: kB}
        idxs = {}
        dloc = {}
        for k, nm_i, nm_d in ((0, "idxA", "dlA"), (1, "idxB", "dlB")):
            w = m[nm_i][:16]
            idxs[k] = w.T.reshape(-1)
            dloc[k] = m[nm_d].astype(np.float32)
        W1b = m["auxW"][:, 0:128].astype(np.float32)
        W2b = m["auxW"][:, 128:256].astype(np.float32)
        rcp = m["rcp"].astype(np.float32)[0]
        outT = np.zeros((D, NPC), np.float32)
        for j in range(NB):
            agg = np.zeros((P, P), np.float32)
            for k in (0, 1):
                for t in range(int(kS[k][j])):
                    ch = int(start[k][j]) + t
                    gi = idxs[k][ch * P : (ch + 1) * P].astype(np.int64)
                    G = tabs[k][gi].astype(np.float32)     # [128 edges, D]
                    O = (dloc[k][:, ch, None] == np.arange(P)[None, :]).astype(
                        np.float32
                    )
                    agg += G.T @ O
            hNj = (agg * rcp[None, j * P : (j + 1) * P]).astype(BF16).astype(np.float32)
            hTj = m["hT"][:, j * P : (j + 1) * P].astype(np.float32)
            po = W1b.T @ hTj + W2b.T @ hNj
            outT[:, j * P : (j + 1) * P] = (po + m["auxB"]).astype(BF16)
        outs.append({"outT": outT.astype(BF16)})
    return _assemble(outs, meta)


if __name__ == "__main__":
    # quick host-side validation against the reference math
    rng = np.random.default_rng(0)
    h = rng.standard_normal((N_NODES, D), dtype=np.float32)
    src = rng.integers(0, N_NODES, N_EDGES)
    dst = rng.integers(0, N_NODES, N_EDGES)
    W1 = rng.standard_normal((D, D), dtype=np.float32) * 0.1
    b1 = rng.standard_normal(D, dtype=np.float32) * 0.1
    W2 = rng.standard_normal((D, D), dtype=np.float32) * 0.1
    b2 = rng.standard_normal(D, dtype=np.float32) * 0.1

    msgs_sum = np.zeros((N_NODES, D), np.float32)
    np.add.at(msgs_sum, dst, h[src])
    deg = np.bincount(dst, minlength=N_NODES).astype(np.float32)
    hN = msgs_sum / np.maximum(deg, 1.0)[:, None]
    ref = h @ W1 + b1 + hN @ W2 + b2

    got = _sim(h, src, dst, W1, b1, W2, b2)
    err = np.linalg.norm(got - ref) / np.linalg.norm(ref)
    print("sim rel err (norm):", err)
    print("sim max abs err:", np.abs(got - ref).max())


# revision 12
# speedup vs baseline: 1.2685x; 1.0683x over previous
"""GNN message-passing (copy_u -> segment mean -> two GEMMs) on 8 trn2 NeuronCores.

Strategy (1D dst partitioning, per sharding hint):
  - Nodes are split into 392 blocks of 128; blocks are dealt to the 8 cores
    sorted by in-edge chunk count so per-position chunk counts (which the
    SPMD program must fix uniformly) stay tight. All in-edges of a node live
    on its owner core.
  - h is shipped in bf16. Per block, edges are gathered 128 at a time from
    HBM via dma_gather (256B rows). int16 gather indices only reach 32768
    rows, so h is shipped as two tables (rows [0,32767) and [32767,50000)),
    each with a trailing all-zero row used for pad slots.
  - Aggregation per 128-edge chunk: psum[feat,dst] += G_chunk.T @ O_chunk on
    the PE (bf16), with O[e,d] = (dst_local[e]==d) a pure 0/1 one-hot. One
    DVE tensor_tensor(is_equal) builds one-hots for OHB chunks at once in an
    interleaved layout oh[e, d*OHB+c] (keeps every operand's innermost dim
    stride-1 so the DVE 2x mode applies); each matmul reads its chunk as a
    stride-OHB view. Batching cuts DVE instruction count ~8x, which removes
    the per-chunk semaphore round-trips that paced the pipeline drain.
  - The mean division: hN = psum * recip[dst] applied during the DVE PSUM
    evacuation as a tensor_tensor multiply against a [128, NPC] recip-
    broadcast tile built once on GPSIMD (partition_broadcast of a host
    [1, NPC] row).
  - Per block: psum_out[dout,node] = W1.T @ hT_blk + W2.T @ hNT_blk
    (weights stationary, bf16), bias added during the ScalarE PSUM
    evacuation (activation Identity with per-partition fp32 bias). Output is
    written feature-major in bf16, two blocks per DMA (512B descriptors),
    and transposed back on the host.

Self-contained: only needs numpy + the concourse stack at /opt/trn_rl_repo.
"""

import sys

if "/opt/trn_rl_repo" not in sys.path:
    sys.path.insert(0, "/opt/trn_rl_repo")

import numpy as np
from contextlib import ExitStack

import ml_dtypes

BF16 = ml_dtypes.bfloat16

N_NODES = 50000
N_EDGES = 800000
D = 128
P = 128
NCORES = 8
NB = 49                      # blocks per core
NPC = NB * P                 # nodes per core (6272)
SPLIT = 32767                # table A holds h rows [0, SPLIT); idx SPLIT = zero row
B_ROWS = N_NODES - SPLIT     # 17233 data rows in table B
BCH_A = 32                   # chunks per gather batch (4096 idxs)
BCH_B = 32                   # chunks per gather batch, B stream
OHB = 8                      # chunks per one-hot build
PAD_DLOC = 200.0             # dst_local sentinel -> one-hot column of zeros


def _prep(h, src, dst, W1, b1, W2, b2):
    """Host-side scheduling. Returns (in_maps, meta)."""
    src = np.asarray(src).astype(np.int64)
    dst = np.asarray(dst).astype(np.int64)
    h = np.asarray(h, dtype=np.float32)

    deg = np.bincount(dst, minlength=N_NODES).astype(np.float32)
    recip = (1.0 / np.maximum(deg, 1.0)).astype(np.float32)

    h16 = h.astype(BF16)
    tabA = np.vstack([h16[:SPLIT], np.zeros((1, D), BF16)])          # [32768, D]
    tabB = np.vstack([h16[SPLIT:], np.zeros((1, D), BF16)])          # [17234, D]

    gblk = dst // P                                   # global block id, 0..391
    slot = dst % P
    cls = (src >= SPLIT).astype(np.int64)

    # per-global-block class counts -> balanced assignment of blocks to
    # (core, position): sort blocks by chunk needs, deal 8 consecutive per
    # position so the per-position max over cores is tight.
    gcnt = np.bincount(gblk * 2 + cls, minlength=NCORES * NB * 2).reshape(NCORES * NB, 2)
    need = np.ceil(gcnt / P).astype(np.int64)         # [392, 2]
    blk_order = np.lexsort((need[:, 1], need[:, 0]))[::-1]
    asg = blk_order.reshape(NB, NCORES)               # asg[j, c] = global block
    # interleave big/small positions so the pipeline tail isn't a train of
    # tiny blocks (their fixed per-block latency starves the DMA stream)
    riffle = []
    lo, hi = 0, NB - 1
    while lo <= hi:
        riffle.append(lo)
        if lo != hi:
            riffle.append(hi)
        lo += 1
        hi -= 1
    asg = asg[np.array(riffle)]
    # owner/position lookup per global block
    pos_of = np.empty(NCORES * NB, np.int64)
    core_of = np.empty(NCORES * NB, np.int64)
    for j in range(NB):
        for c in range(NCORES):
            g = asg[j, c]
            pos_of[g] = j
            core_of[g] = c
    owner = core_of[gblk]
    blk = pos_of[gblk]

    # counts per (core, position, class)
    key = ((owner * NB) + blk) * 2 + cls
    cnt = np.bincount(key, minlength=NCORES * NB * 2).reshape(NCORES, NB, 2)

    # program chunk counts per block position (uniform across cores)
    kA = np.maximum(np.ceil(cnt[:, :, 0] / P).astype(np.int64).max(axis=0), 1)  # [NB]
    kB = np.ceil(cnt[:, :, 1] / P).astype(np.int64).max(axis=0)                 # [NB]
    startA = np.concatenate([[0], np.cumsum(kA)])
    startB = np.concatenate([[0], np.cumsum(kB)])
    nchA, nchB = int(startA[-1]), int(startB[-1])
    nchB = max(nchB, 1)  # keep tensor shapes nonzero if no class-B edges exist
    SA, SB = nchA * P, nchB * P
    # dl tensors padded to a multiple of OHB so batched one-hot builds stay
    # in bounds (pad columns hold PAD_DLOC -> all-zero one-hots, never used)
    nchAp = -(-nchA // OHB) * OHB
    nchBp = -(-nchB // OHB) * OHB

    # per-edge slot position within its (core, blk, cls) group; edges are
    # ordered by src within each group so every gather batch reads ascending
    # HBM addresses (row-buffer/bank friendliness; any order is correct)
    order = np.lexsort((src, cls, blk, owner))
    e_sorted = order
    grp = key[e_sorted]
    # position within group
    grp_change = np.concatenate([[0], np.cumsum(grp[1:] != grp[:-1])])
    first_of_grp = np.concatenate([[0], np.flatnonzero(grp[1:] != grp[:-1]) + 1])
    pos = np.arange(len(e_sorted)) - first_of_grp[grp_change]

    # interleaved iota constant: value d at column d*OHB + c
    iota_int = np.repeat(np.arange(P, dtype=np.float32), OHB)[None, :]
    iota_int = np.tile(iota_int, (P, 1)).astype(BF16)               # [128, 128*OHB]

    in_maps = []
    start = {0: startA, 1: startB}
    for c in range(NCORES):
        idxs = {0: np.full(SA, SPLIT, np.int64), 1: np.full(SB, B_ROWS, np.int64)}
        dloc = {
            0: np.full(nchAp * P, PAD_DLOC, np.float32),
            1: np.full(nchBp * P, PAD_DLOC, np.float32),
        }

        mask_c = owner[e_sorted] == c
        for k in (0, 1):
            m = mask_c & (cls[e_sorted] == k)
            es = e_sorted[m]
            ps = pos[m]
            gslot = start[k][blk[es]] * P + ps
            v = src[es] - (SPLIT if k else 0)
            idxs[k][gslot] = v
            dloc[k][gslot] = slot[es]

        def idx_layout(v):
            w = v.astype(np.int16).reshape(-1, 16).T        # [16, S/16]
            return np.tile(w, (8, 1)).copy()                # [128, S/16]

        def col_layout(v):
            return v.reshape(-1, P).T.copy()                # [128, nch]

        hT = np.zeros((D, NPC), np.float32)
        rcp = np.zeros((1, NPC), np.float32)
        for j in range(NB):
            g = asg[j, c]
            lo = g * P
            hi = min(lo + P, N_NODES)
            if hi > lo:
                hT[:, j * P : j * P + (hi - lo)] = h[lo:hi].T
                rcp[0, j * P : j * P + (hi - lo)] = recip[lo:hi]

        auxW = np.concatenate(
            [np.asarray(W1, np.float32), np.asarray(W2, np.float32)], axis=1
        ).astype(BF16)  # [128, 256]
        auxB = (np.asarray(b1, np.float32) + np.asarray(b2, np.float32))[:, None]

        in_maps.append(
            {
                "tabA": tabA,
                "tabB": tabB,
                "idxA": idx_layout(idxs[0]),
                "idxB": idx_layout(idxs[1]),
                "dlA": col_layout(dloc[0]).astype(BF16),
                "dlB": col_layout(dloc[1]).astype(BF16),
                "hT": hT.astype(BF16),
                "rcp": rcp.astype(BF16),
                "ioI": iota_int,
                "auxW": auxW,
                "auxB": auxB.astype(np.float32),
            }
        )

    meta = dict(
        kA=kA,
        kB=kB,
        startA=startA,
        startB=startB,
        nchA=nchA,
        nchB=nchB,
        nchAp=nchAp,
        nchBp=nchBp,
        asg=asg,
    )
    return in_maps, meta


def _build(meta):
    import concourse.bacc as bacc
    import concourse.mybir as mybir
    import concourse.tile as tile

    kA, kB = meta["kA"], meta["kB"]
    startA, startB = meta["startA"], meta["startB"]
    nchA, nchB = meta["nchA"], meta["nchB"]
    nchAp, nchBp = meta["nchAp"], meta["nchBp"]
    f32 = mybir.dt.float32
    bf16 = mybir.dt.bfloat16

    nc = bacc.Bacc("TRN2", target_bir_lowering=False, debug=False, num_devices=NCORES)
    tabA = nc.declare_dram_parameter("tabA", [SPLIT + 1, D], bf16, isOutput=False)
    tabB = nc.declare_dram_parameter("tabB", [B_ROWS + 1, D], bf16, isOutput=False)
    idxA = nc.declare_dram_parameter("idxA", [P, nchA * 8], mybir.dt.int16, isOutput=False)
    idxB = nc.declare_dram_parameter("idxB", [P, nchB * 8], mybir.dt.int16, isOutput=False)
    dlA = nc.declare_dram_parameter("dlA", [P, nchAp], bf16, isOutput=False)
    dlB = nc.declare_dram_parameter("dlB", [P, nchBp], bf16, isOutput=False)
    hT = nc.declare_dram_parameter("hT", [D, NPC], bf16, isOutput=False)
    rcp = nc.declare_dram_parameter("rcp", [1, NPC], bf16, isOutput=False)
    ioI = nc.declare_dram_parameter("ioI", [P, P * OHB], bf16, isOutput=False)
    auxW = nc.declare_dram_parameter("auxW", [P, 256], bf16, isOutput=False)
    auxB = nc.declare_dram_parameter("auxB", [P, 1], f32, isOutput=False)
    outT = nc.declare_dram_parameter("outT", [D, NPC], bf16, isOutput=True)

    tabs = {0: tabA, 1: tabB}
    idx_d = {0: idxA, 1: idxB}
    dl_d = {0: dlA, 1: dlB}
    nch = {0: nchA, 1: nchB}
    nchp = {0: nchAp, 1: nchBp}
    bch = {0: BCH_A, 1: BCH_B}

    with tile.TileContext(nc) as tc, ExitStack() as ctx:
        consts = ctx.enter_context(tc.tile_pool(name="consts", bufs=1))
        gA_pool = ctx.enter_context(tc.tile_pool(name="gA", bufs=5))
        gB_pool = ctx.enter_context(tc.tile_pool(name="gB", bufs=4))
        ohA_pool = ctx.enter_context(tc.tile_pool(name="ohA", bufs=10))
        ohB_pool = ctx.enter_context(tc.tile_pool(name="ohB", bufs=6))
        hN_pool = ctx.enter_context(tc.tile_pool(name="hNp", bufs=4))
        out_pool = ctx.enter_context(tc.tile_pool(name="outp", bufs=8))
        psA = ctx.enter_context(tc.tile_pool(name="psA", bufs=5, space="PSUM"))
        psO = ctx.enter_context(tc.tile_pool(name="psO", bufs=3, space="PSUM"))

        # constant loads; head slices land first so the gather/onehot pipeline
        # can start while the bulk loads behind it.
        HEAD = 2 * BCH_A
        idx_t = {}
        dl_t = {}
        for k in (0, 1):
            it = consts.tile([P, nch[k] * 8], mybir.dt.int16, name=f"idx{k}")
            h8 = min(HEAD * 8, nch[k] * 8)
            nc.sync.dma_start(it[:, :h8], idx_d[k][:, :h8])
            if nch[k] * 8 > h8:
                nc.sync.dma_start(it[:, h8:], idx_d[k][:, h8:])
            idx_t[k] = it
            dt_ = consts.tile([P, nchp[k]], bf16, name=f"dl{k}")
            hd = min(HEAD, nchp[k])
            nc.sync.dma_start(dt_[:, :hd], dl_d[k][:, :hd])
            if nchp[k] > hd:
                nc.sync.dma_start(dt_[:, hd:], dl_d[k][:, hd:])
            dl_t[k] = dt_
        auxW_t = consts.tile([P, 256], bf16)
        nc.sync.dma_start(auxW_t[:], auxW[:])
        auxB_t = consts.tile([P, 1], f32)
        nc.sync.dma_start(auxB_t[:], auxB[:])
        ioI_t = consts.tile([P, P * OHB], bf16)
        nc.sync.dma_start(ioI_t[:], ioI[:])
        rcp_t = consts.tile([1, NPC], bf16)
        nc.sync.dma_start(rcp_t[:], rcp[:])
        hT_all = consts.tile([P, NPC], bf16)
        nc.sync.dma_start(hT_all[:], hT[:])
        w1_s = auxW_t[:, 0:128]
        w2_s = auxW_t[:, 128:256]
        bias_s = auxB_t[:, 0:1]

        # recip broadcast across partitions, built once on GPSIMD
        rb_t = consts.tile([P, NPC], bf16)
        nc.gpsimd.partition_broadcast(rb_t[:], rcp_t[:])

        g_pool = {0: gA_pool, 1: gB_pool}
        oh_pool = {0: ohA_pool, 1: ohB_pool}
        g_state = {0: (-1, None), 1: (-1, None)}
        oh_state = {0: (-1, None), 1: (-1, None)}

        def chunk_ap(k, j):
            """AP of gathered G chunk j of stream k (emits batch gather on demand)."""
            b, off = divmod(j, bch[k])
            cur, tile_ = g_state[k]
            if b != cur:
                nchunks = min(bch[k], nch[k] - b * bch[k])
                gt = g_pool[k].tile([P, bch[k] * D], bf16, name=f"g{k}")
                ni = nchunks * P
                nc.gpsimd.dma_gather(
                    gt[:, : nchunks * D].rearrange("p (c e) -> p c e", e=D),
                    tabs[k][:, :],
                    idx_t[k][:, b * bch[k] * 8 : b * bch[k] * 8 + (ni // 16)],
                    ni,
                    ni,
                    D,
                    single_packet=False,
                )
                g_state[k] = (b, gt)
                tile_ = gt
            return tile_[:, off * D : (off + 1) * D]

        def oh_ap(k, j):
            """AP of the one-hot for chunk j of stream k ([e, d] stride-OHB view;
            emits the batched is_equal build on demand)."""
            b, ci = divmod(j, OHB)
            cur, tile_ = oh_state[k]
            if b != cur:
                ot = oh_pool[k].tile([P, P * OHB], bf16, name=f"oh{k}")
                in1 = (
                    dl_t[k][:, b * OHB : (b + 1) * OHB]
                    .rearrange("p (x c) -> p x c", x=1)
                    .broadcast_to([P, P, OHB])
                )
                nc.vector.tensor_tensor(
                    out=ot[:].rearrange("p (d c) -> p d c", c=OHB),
                    in0=ioI_t[:].rearrange("p (d c) -> p d c", c=OHB),
                    in1=in1,
                    op=mybir.AluOpType.is_equal,
                )
                oh_state[k] = (b, ot)
                tile_ = ot
            return tile_[:].rearrange("p (d c) -> p c d", c=OHB)[:, ci, :]

        for j in range(NB):
            agg = psA.tile([P, P], f32)
            plan = [(0, int(startA[j]) + t) for t in range(int(kA[j]))] + [
                (1, int(startB[j]) + t) for t in range(int(kB[j]))
            ]
            for i, (k, ch) in enumerate(plan):
                g_ap = chunk_ap(k, ch)
                nc.tensor.matmul(
                    agg[:],
                    lhsT=g_ap,
                    rhs=oh_ap(k, ch),
                    start=(i == 0),
                    stop=(i == len(plan) - 1),
                )

            # evacuate with the mean division folded in: hN = agg * recip[dst]
            hN_t = hN_pool.tile([P, P], bf16)
            nc.vector.tensor_tensor(
                out=hN_t[:],
                in0=agg[:],
                in1=rb_t[:, j * P : (j + 1) * P],
                op=mybir.AluOpType.mult,
            )

            hT_slice = hT_all[:, j * P : (j + 1) * P]

            po = psO.tile([P, P], f32)
            nc.tensor.matmul(po[:], lhsT=w1_s, rhs=hT_slice, start=True, stop=False)
            nc.tensor.matmul(po[:], lhsT=w2_s, rhs=hN_t[:], start=False, stop=True)

            if j % 2 == 0:
                out_t2 = out_pool.tile([P, 2 * P], bf16)
            out_sl = out_t2[:, 0:P] if j % 2 == 0 else out_t2[:, P : 2 * P]
            nc.scalar.activation(
                out_sl, po[:], mybir.ActivationFunctionType.Identity, bias=bias_s
            )
            if j % 2 == 1:
                nc.sync.dma_start(outT[:, (j - 1) * P : (j + 1) * P], out_t2[:])
            elif j == NB - 1:
                nc.sync.dma_start(outT[:, j * P : (j + 1) * P], out_t2[:, 0:P])

    nc.finalize()
    return nc


def kernel(h, src, dst, W1, b1, W2, b2):
    from concourse.bass_utils import run_bass_kernel_spmd

    in_maps, meta = _prep(h, src, dst, W1, b1, W2, b2)
    nc = _build(meta)
    res = run_bass_kernel_spmd(nc, in_maps, list(range(NCORES))).results
    return _assemble(res, meta)


def _assemble(res, meta):
    asg = meta["asg"]
    out = np.zeros((NCORES * NPC, D), np.float32)
    for j in range(NB):
        for c in range(NCORES):
            g = asg[j, c]
            out[g * P : (g + 1) * P] = (
                res[c]["outT"][:, j * P : (j + 1) * P].astype(np.float32).T
            )
    return out[:N_NODES].astype(np.float32)


def _sim(h, src, dst, W1, b1, W2, b2):
    """Numpy simulation of the exact device program (bookkeeping check)."""
    in_maps, meta = _prep(h, src, dst, W1, b1, W2, b2)
    kA, kB = meta["kA"], meta["kB"]
    startA, startB = meta["startA"], meta["startB"]
    outs = []
    for c in range(NCORES):
        m = in_maps[c]
        tabs = {0: m["tabA"], 1: m["tabB"]}
        start = {0: startA, 1: startB}
        kS = {0: kA, 1: kB}
        idxs = {}
        dloc = {}
        for k, nm_i, nm_d in ((0, "idxA", "dlA"), (1, "idxB", "dlB")):
            w = m[nm_i][:16]
            idxs[k] = w.T.reshape(-1)
            dloc[k] = m[nm_d].astype(np.float32)
        W1b = m["auxW"][:, 0:128].astype(np.float32)
        W2b = m["auxW"][:, 128:256].astype(np.float32)
        rcp = m["rcp"].astype(np.float32)[0]
        outT = np.zeros((D, NPC), np.float32)
        for j in range(NB):
            agg = np.zeros((P, P), np.float32)
            for k in (0, 1):
                for t in range(int(kS[k][j])):
                    ch = int(start[k][j]) + t
                    gi = idxs[k][ch * P : (ch + 1) * P].astype(np.int64)
                    G = tabs[k][gi].astype(np.float32)     # [128 edges, D]
                    O = (dloc[k][:, ch, None] == np.arange(P)[None, :]).astype(
                        np.float32
                    )
                    agg += G.T @ O
            hNj = (agg * rcp[None, j * P : (j + 1) * P]).astype(BF16).astype(np.float32)
            hTj = m["hT"][:, j * P : (j + 1) * P].astype(np.float32)
            po = W1b.T @ hTj + W2b.T @ hNj
            outT[:, j * P : (j + 1) * P] = (po + m["auxB"]).astype(BF16)
        outs.append({"outT": outT.astype(BF16)})
    return _assemble(outs, meta)


if __name__ == "__main__":
    # quick host-side validation against the reference math
    rng = np.random.default_rng(0)
    h = rng.standard_normal((N_NODES, D), dtype=np.float32)
    src = rng.integers(0, N_NODES, N_EDGES)
    dst = rng.integers(0, N_NODES, N_EDGES)
    W1 = rng.standard_normal((D, D), dtype=np.float32) * 0.1
    b1 = rng.standard_normal(D, dtype=np.float32) * 0.1
    W2 = rng.standard_normal((D, D), dtype=np.float32) * 0.1
    b2 = rng.standard_normal(D, dtype=np.float32) * 0.1

    msgs_sum = np.zeros((N_NODES, D), np.float32)
    np.add.at(msgs_sum, dst, h[src])
    deg = np.bincount(dst, minlength=N_NODES).astype(np.float32)
    hN = msgs_sum / np.maximum(deg, 1.0)[:, None]
    ref = h @ W1 + b1 + hN @ W2 + b2

    got = _sim(h, src, dst, W1, b1, W2, b2)
    err = np.linalg.norm(got - ref) / np.linalg.norm(ref)
    print("sim rel err (norm):", err)
    print("sim max abs err:", np.abs(got - ref).max())
